# revision 3
# baseline (speedup 1.0000x reference)
"""Trainium2 Bass kernel for the MetricLearning pairwise loss.

Math (reference):
    d2[i,j] = max(||x_i||^2 + ||x_j||^2 - 2 x_i.x_j, EPS)
    a = d2/(2k)/sigma^2 ; b = d2/(2k)/omega^2 ; coeff = k/2-1
    per_pair = same ? (-coeff*log(a) + a/2) : (coeff*log(b) - b/2)
    loss = sum_{i<j} per_pair

Rewritten per element with L = log(d2), t = x_i.x_j - sq_j/2 (so d2 = -2t + sq_i):
    diff_val/c1 = L + q   where q =  (B/c1)*t + (logB - (B/(2c1))*sq_i)
    same_val/c1 = -L + r  where r = -(A/c1)*t + (-logA + (A/(2c1))*sq_i)
    w = r - q = -((A+B)/c1)*t + (-logA - logB + ((A+B)/(2c1))*sq_i)
    per_pair/c1 = (L + q) + m*(w - 2L)   with m = 1 if labels equal else 0
    loss = c1 * [ sum(L+q) + sum(m*w) - 2*sum(m*L) ]   over the chosen pairs

Sharding: 16 row-blocks of 256. The K16 block-pair graph is oriented so every
core owns one even block (8 cross partners) + one odd block (7 partners) plus
both within-block triangles -> every core runs an IDENTICAL program (SPMD);
all per-core variation lives in the input data (slab permutation).

Per core on device:
    PE   : gram tiles in bf16 (8 K-chunks) + K=2 augmented chunk adding
           -sq_j/2 (hi+lo bf16 split), label row broadcast, final dot.
    ACT  : L = Ln(-2t + sq_i), q/w = Identity(scale*t + bias) (one act table).
    DVE  : three fused tensor_tensor_reduce ops per job (+ diag clamp).
    Pool : label-equality masks, strict-upper-triangle affine_select on diag.
"""

import numpy as np
import ml_dtypes

N = 4096
D = 1024
P = 128
NB = 16          # row blocks
BLK = 256        # rows per block
KC = D // P      # k chunks (8)
NCORES = 8

SIGMA = 0.2
OMEGA = 1.0
K_F = float(N)
C1 = K_F / 2.0 - 1.0                      # 2047
A_C = 1.0 / (2.0 * K_F * SIGMA * SIGMA)   # 1/327.68
B_C = 1.0 / (2.0 * K_F * OMEGA * OMEGA)   # 1/8192
LOG_A = float(np.log(A_C))
LOG_B = float(np.log(B_C))
SCALE_Q = B_C / C1
SCALE_W = -(A_C + B_C) / C1
EPS_D2 = 1e-3   # clamp floor for the (masked-out) diagonal; real d2 >= ~1500

# job := (lhs_slab in {0,1}, unit u in {0,1}, col_lo in slots*BLK, width, diag)
JOBS = []
for _u in (0, 1):
    JOBS.append((0, _u, 0, 256, True))
    JOBS.append((1, _u, 256, 256, True))
for _u in (0, 1):
    for _g in ((256, 512), (768, 512), (1280, 512), (1792, 512)):
        JOBS.append((0, _u, _g[0], _g[1], False))
    for _g in ((2304, 512), (2816, 512), (3328, 512), (3840, 256)):
        JOBS.append((1, _u, _g[0], _g[1], False))
NJOBS = len(JOBS)  # 20


def _partners(d):
    """Block orientation: edge {i,j} (i<j) owned by i if i+j odd else j."""
    l0, l1 = 2 * d, 2 * d + 1
    p8 = [j for j in range(l0 + 1, NB) if j % 2 == 1] + \
         [i for i in range(0, l0) if i % 2 == 0]
    p7 = [j for j in range(l1 + 1, NB) if j % 2 == 0] + \
         [i for i in range(0, l1) if i % 2 == 1]
    assert len(p8) == 8 and len(p7) == 7 and l1 in p8
    return l0, l1, p8, p7


def _core_slabs(d):
    """Slot -> block id (16 slots). Slot0=own even, slot1=own odd."""
    l0, l1, p8, p7 = _partners(d)
    slabs = [l0, l1] + [p for p in p8 if p != l1] + list(p7)
    assert len(slabs) == NB and len(set(slabs)) == NB
    return slabs


_PROG_CACHE = {}


def _build_program():
    if "nc" in _PROG_CACHE:
        return _PROG_CACHE["nc"]
    import concourse.bass as bass  # noqa: F401
    import concourse.bacc as bacc
    import concourse.mybir as mybir
    import concourse.tile as tile

    F32 = mybir.dt.float32
    BF16 = mybir.dt.bfloat16
    AF = mybir.ActivationFunctionType
    ALU = mybir.AluOpType

    nc = bacc.Bacc("TRN2", target_bir_lowering=False, debug=False,
                   num_devices=NCORES)
    xtp_d = nc.dram_tensor("xtp", [NB, D, BLK], BF16, kind="ExternalInput").ap()
    aug_d = nc.dram_tensor("aug", [2, N], BF16, kind="ExternalInput").ap()
    lab_d = nc.dram_tensor("lab", [1, N], BF16, kind="ExternalInput").ap()
    rowd_d = nc.dram_tensor("rowd", [P, 4 * 5], F32, kind="ExternalInput").ap()
    out_d = nc.dram_tensor("out", [1, 1], F32, kind="ExternalOutput").ap()

    with tile.TileContext(nc) as tc:
        with (
            tc.tile_pool(name="persist", bufs=1) as persist,
            tc.tile_pool(name="scratch", bufs=3) as scratch,
            tc.tile_pool(name="dscratch", bufs=2) as dscratch,
            tc.tile_pool(name="junk", bufs=4) as junkpool,
            tc.tile_pool(name="psum", bufs=5, space="PSUM") as psum,
            tc.tile_pool(name="psumlab", bufs=2, space="PSUM") as psumlab,
            tc.tile_pool(name="psumloss", bufs=1, space="PSUM") as psumloss,
        ):
            xall = persist.tile([P, KC, NB, BLK], BF16, tag="xall")
            labb = persist.tile([P, N], F32, tag="labb")
            labr = persist.tile([1, N], BF16, tag="labr")
            augs = persist.tile([2, N], BF16, tag="augs")
            rd = persist.tile([P, 4 * 5], F32, tag="rd")
            ones2 = persist.tile([2, P], BF16, tag="ones2")
            ones1f = persist.tile([P, 1], F32, tag="ones1f")
            acc = persist.tile([P, 3 * NJOBS], F32, tag="acc")
            tot = persist.tile([P, 1], F32, tag="tot")
            lossb = persist.tile([1, 1], F32, tag="lossb")

            # input DMAs; slabs in consumption order
            nc.sync.dma_start(out=rd[:], in_=rowd_d[:])
            nc.sync.dma_start(out=augs[:], in_=aug_d[:])
            nc.sync.dma_start(out=labr[:], in_=lab_d[:])
            for s in range(NB):
                nc.sync.dma_start(
                    out=xall[:, :, s, :],
                    in_=xtp_d[s].rearrange("(kc p) n -> p kc n", p=P),
                )

            nc.gpsimd.memset(ones2[:], 1.0)
            nc.gpsimd.memset(ones1f[:], C1)  # folds the global c1 factor

            # broadcast label row across partitions via PE (ones[1,128]^T @ lab)
            for j in range(N // 512):
                pl = psumlab.tile([P, 512], F32, tag="pl")
                nc.tensor.matmul(pl[:], ones2[0:1, :],
                                 labr[0:1, 512 * j:512 * (j + 1)],
                                 start=True, stop=True)
                nc.scalar.activation(labb[:, 512 * j:512 * (j + 1)], pl[:],
                                     AF.Copy)

            for ji, (ls, u, clo, wid, diag) in enumerate(JOBS):
                g = 2 * ls + u
                sq_ap = rd[:, 5 * g + 0:5 * g + 1]
                bq_ap = rd[:, 5 * g + 1:5 * g + 2]
                bw_ap = rd[:, 5 * g + 2:5 * g + 3]
                lb_ap = rd[:, 5 * g + 3:5 * g + 4]
                th_ap = rd[:, 5 * g + 4:5 * g + 5]

                t = psum.tile([P, wid], F32, tag="gram")
                ns = wid // BLK
                s0 = clo // BLK
                for kc in range(KC):
                    nc.tensor.matmul(
                        t[:],
                        xall[:, kc, ls, 128 * u:128 * (u + 1)],
                        xall[:, kc, s0:s0 + ns, :],
                        start=(kc == 0), stop=False,
                    )
                nc.tensor.matmul(t[:], ones2[:, :],
                                 augs[:, clo:clo + wid],
                                 start=False, stop=True)

                m = scratch.tile([P, wid], F32, tag="m")
                nc.gpsimd.tensor_scalar(m[:], labb[:, clo:clo + wid],
                                        lb_ap, None, ALU.is_equal)

                Lt = scratch.tile([P, wid], F32, tag="L")
                qt = scratch.tile([P, wid], F32, tag="q")
                wt = scratch.tile([P, wid], F32, tag="w")
                if diag:
                    t2 = dscratch.tile([P, wid], F32, tag="t2")
                    nc.vector.tensor_scalar(t2[:], t[:], th_ap, None, ALU.min)
                    nc.scalar.activation(Lt[:], t2[:], AF.Ln,
                                         bias=sq_ap, scale=-2.0)
                else:
                    nc.scalar.activation(Lt[:], t[:], AF.Ln,
                                         bias=sq_ap, scale=-2.0)
                nc.scalar.activation(qt[:], t[:], AF.Identity,
                                     bias=bq_ap, scale=SCALE_Q)
                nc.scalar.activation(wt[:], t[:], AF.Identity,
                                     bias=bw_ap, scale=SCALE_W)

                if diag:
                    # keep strictly-upper (col > 128*u + p) else 0
                    Lp = dscratch.tile([P, wid], F32, tag="Lp")
                    qp = dscratch.tile([P, wid], F32, tag="qp")
                    wp = dscratch.tile([P, wid], F32, tag="wp")
                    for src, dst in ((Lt, Lp), (qt, qp), (wt, wp)):
                        nc.gpsimd.affine_select(
                            out=dst[:], in_=src[:],
                            compare_op=ALU.is_gt, fill=0.0,
                            base=-128 * u, channel_multiplier=-1,
                            pattern=[[1, wid]],
                        )
                    Lt, qt, wt = Lp, qp, wp

                # accumulate (L+q), m*w, -2*m*L; the c1 factor rides on the
                # final ones-vector of the cross-partition dot.
                j1 = junkpool.tile([P, wid], F32, tag="junk")
                j2 = junkpool.tile([P, wid], F32, tag="junk")
                j3 = junkpool.tile([P, wid], F32, tag="junk")
                nc.vector.scalar_tensor_tensor(
                    out=j1[:], in0=Lt[:], scalar=0.0, in1=qt[:],
                    op0=ALU.add, op1=ALU.add,
                    accum_out=acc[:, 3 * ji + 0:3 * ji + 1])
                nc.vector.scalar_tensor_tensor(
                    out=j2[:], in0=m[:], scalar=1.0, in1=wt[:],
                    op0=ALU.mult, op1=ALU.mult,
                    accum_out=acc[:, 3 * ji + 1:3 * ji + 2])
                nc.vector.scalar_tensor_tensor(
                    out=j3[:], in0=Lt[:], scalar=-2.0, in1=m[:],
                    op0=ALU.mult, op1=ALU.mult,
                    accum_out=acc[:, 3 * ji + 2:3 * ji + 3])

            nc.vector.tensor_reduce(tot[:], acc[:], axis=mybir.AxisListType.X,
                                    op=ALU.add)
            pls = psumloss.tile([1, 1], F32, tag="pls")
            nc.tensor.matmul(pls[:], tot[:], ones1f[:], start=True, stop=True)
            nc.scalar.activation(lossb[:], pls[:], AF.Copy)
            nc.sync.dma_start(out=out_d[:], in_=lossb[:])

    nc.compile()
    _PROG_CACHE["nc"] = nc
    return nc


def _host_inputs(outputs, labels):
    """Build per-core input dicts. outputs [N,D] f32, labels [N] int."""
    x = np.asarray(outputs, dtype=np.float32)
    lab = np.asarray(labels)
    assert x.shape == (N, D)
    xt_bf = np.ascontiguousarray(x.T).astype(ml_dtypes.bfloat16)   # [D, N]
    sq = (x.astype(np.float64) ** 2).sum(axis=1)                   # [N]

    neg_half = -0.5 * sq
    hi = neg_half.astype(ml_dtypes.bfloat16)
    lo = (neg_half - hi.astype(np.float64)).astype(ml_dtypes.bfloat16)
    lab_f = lab.astype(np.float64)

    in_maps = []
    for d in range(NCORES):
        slabs = _core_slabs(d)
        cols = np.concatenate(
            [np.arange(b * BLK, (b + 1) * BLK) for b in slabs])
        xtp = np.ascontiguousarray(
            xt_bf[:, cols].reshape(D, NB, BLK).transpose(1, 0, 2))
        aug = np.stack([hi[cols], lo[cols]])                       # [2, N]
        labrow = lab_f[cols].astype(ml_dtypes.bfloat16)[None, :]   # [1, N]

        rowd = np.zeros((P, 4 * 5), dtype=np.float64)
        for g, (slab, u) in enumerate(((0, 0), (0, 1), (1, 0), (1, 1))):
            rows = slabs[slab] * BLK + 128 * u + np.arange(P)
            sqr = sq[rows]
            rowd[:, 5 * g + 0] = sqr
            rowd[:, 5 * g + 1] = LOG_B - (B_C / (2 * C1)) * sqr
            rowd[:, 5 * g + 2] = (-LOG_A - LOG_B
                                  + ((A_C + B_C) / (2 * C1)) * sqr)
            rowd[:, 5 * g + 3] = lab_f[rows]
            rowd[:, 5 * g + 4] = (sqr - EPS_D2) / 2.0
        in_maps.append({
            "xtp": xtp,
            "aug": np.ascontiguousarray(aug),
            "lab": np.ascontiguousarray(labrow),
            "rowd": rowd.astype(np.float32),
        })
    return in_maps


def kernel(**inputs):
    from concourse.bass_utils import run_bass_kernel_spmd
    nc = _build_program()
    in_maps = _host_inputs(inputs["outputs"], inputs["labels"])
    res = run_bass_kernel_spmd(nc, in_maps, core_ids=list(range(NCORES)))
    total = np.float64(0.0)
    for r in res.results:
        total += np.float64(r["out"][0, 0])
    return np.asarray(total, dtype=np.float32)


# revision 6
# speedup vs baseline: 2.5850x; 2.5850x over previous
"""Trainium2 Bass kernel for the MetricLearning pairwise loss.

Reference math:
    d2[i,j] = max(||x_i||^2 + ||x_j||^2 - 2 x_i.x_j, EPS)
    a = d2/(2k)/sigma^2 ; b = d2/(2k)/omega^2 ; c1 = k/2-1
    per_pair = same ? (-c1*log(a) + a/2) : (c1*log(b) - b/2)
    loss = sum_{i<j} per_pair

Per element, with L = log(d2) and t = x_i.x_j - sq_j/2 (so d2 = -2t + sq_i):
    diff_val = c1*L + B*t + c1*bias_q(i),  bias_q = logB - (B/(2c1))*sq_i
    same-diff correction = -2c1*L - (A+B)*t + c1*bias_w(i),
                           bias_w = -logA - logB + ((A+B)/(2c1))*sq_i
    loss = c1*SUM(L) + B*SUM(t)                      [over all pairs]
         - 2c1*SUM_same(L) - (A+B)*SUM_same(t)       [over same-label pairs]
         + c1*(sum_i bias_q(i)*cnt_main(i) + bias_w(i)*cnt_same(i))  [host]

Rows are globally SORTED BY LABEL, so same-label pairs live only within a
block or in the corner between consecutive blocks (label runs < 128 rows).
The main term therefore needs NO label mask at all (ACT accум + one DVE
reduce per tile); the correction runs on 6 small regions per core.

Sharding: 16 row-blocks of 256; the K16 block-pair graph is oriented so
every core owns one even block (8 partners) + one odd block (7 partners)
plus both within-block triangles -> identical SPMD program on all 8 cores,
per-core variation only in input data (slab permutation).
"""

import numpy as np
import ml_dtypes

N = 4096
D = 1024
P = 128
NB = 16          # row blocks
BLK = 256        # rows per block
KC = D // P      # k chunks (8)
NCORES = 8

SIGMA = 0.2
OMEGA = 1.0
K_F = float(N)
C1 = K_F / 2.0 - 1.0                      # 2047
A_C = 1.0 / (2.0 * K_F * SIGMA * SIGMA)   # 1/327.68
B_C = 1.0 / (2.0 * K_F * OMEGA * OMEGA)   # 1/8192
LOG_A = float(np.log(A_C))
LOG_B = float(np.log(B_C))
EPS_D2 = 1e-3   # clamp floor for the (masked-out) diagonal; real d2 >= ~1500

# job := (lhs_slab in {0,1}, unit u in {0,1}, col_lo in slots*BLK, width, diag)
JOBS = []
for _u in (0, 1):
    JOBS.append((0, _u, 0, 256, True))
    JOBS.append((1, _u, 256, 256, True))
for _u in (0, 1):
    for _g in ((256, 512), (768, 512), (1280, 512), (1792, 512)):
        JOBS.append((0, _u, _g[0], _g[1], False))
    for _g in ((2304, 512), (2816, 512), (3328, 512), (3840, 256)):
        JOBS.append((1, _u, _g[0], _g[1], False))
NJOBS = len(JOBS)  # 20

# correction regions: (job_idx, corner?) — diag jobs (0..3) get in-tile
# upper-triangle same-label correction; the two u=1 cross jobs that start
# at slot1 / slot9 get a 128-wide corner correction (consecutive blocks).
DIAG_JOBS = [ji for ji, j in enumerate(JOBS) if j[4]]
CORNER_JOBS = [ji for ji, j in enumerate(JOBS)
               if not j[4] and j[1] == 1 and j[2] in (256, 2304)]
CORNER_W = 128

# acc column map (raw sums; coefficients applied in the final dot)
ACC_W = 64
COL_L = {ji: ji for ji in range(NJOBS)}              # 0..19   coeff c1
COL_T = {ji: 20 + ji for ji in range(NJOBS)}         # 20..39  coeff B
_corr = DIAG_JOBS + CORNER_JOBS
COL_ML = {ji: 40 + k for k, ji in enumerate(_corr)}  # 40..45  coeff -2c1
COL_MT = {ji: 48 + k for k, ji in enumerate(_corr)}  # 48..53  coeff -(A+B)
COEFS = [C1, B_C, -2.0 * C1, -(A_C + B_C)]           # per group of 16 cols
GROUPS = [(0, 20), (20, 40), (40, 46), (48, 54)]


def _partners(d):
    """Block orientation: edge {i,j} (i<j) owned by i if i+j odd else j."""
    l0, l1 = 2 * d, 2 * d + 1
    p8 = [j for j in range(l0 + 1, NB) if j % 2 == 1] + \
         [i for i in range(0, l0) if i % 2 == 0]
    p7 = [j for j in range(l1 + 1, NB) if j % 2 == 0] + \
         [i for i in range(0, l1) if i % 2 == 1]
    assert len(p8) == 8 and len(p7) == 7 and l1 in p8
    return l0, l1, p8, p7


def _core_slabs(d):
    """Slot -> block id (16 slots). slot0=own even, slot1=own odd, and
    slot9 (first partner of the odd block) pinned to block 2d+2 when it
    exists so the consecutive-pair corner lands at a fixed slot."""
    l0, l1, p8, p7 = _partners(d)
    rest8 = [p for p in p8 if p != l1]
    nxt = l1 + 1
    if nxt in p7:
        p7 = [nxt] + [p for p in p7 if p != nxt]
    slabs = [l0, l1] + rest8 + list(p7)
    assert len(slabs) == NB and len(set(slabs)) == NB
    return slabs


_PROG_CACHE = {}


def _build_program():
    if "nc" in _PROG_CACHE:
        return _PROG_CACHE["nc"]
    import concourse.bass as bass  # noqa: F401
    import concourse.bacc as bacc
    import concourse.mybir as mybir
    import concourse.tile as tile

    F32 = mybir.dt.float32
    BF16 = mybir.dt.bfloat16
    AF = mybir.ActivationFunctionType
    ALU = mybir.AluOpType

    nc = bacc.Bacc("TRN2", target_bir_lowering=False, debug=False,
                   num_devices=NCORES)
    xtp_d = nc.dram_tensor("xtp", [NB, D, BLK], BF16, kind="ExternalInput").ap()
    aug_d = nc.dram_tensor("aug", [2, N], BF16, kind="ExternalInput").ap()
    lab_d = nc.dram_tensor("lab", [1, 640], BF16, kind="ExternalInput").ap()
    rowd_d = nc.dram_tensor("rowd", [P, 4 * 3], F32, kind="ExternalInput").ap()
    coef_d = nc.dram_tensor("coef", [4, 1], F32, kind="ExternalInput").ap()
    out_d = nc.dram_tensor("out", [1, 1], F32, kind="ExternalOutput").ap()

    with tile.TileContext(nc) as tc:
        with (
            tc.tile_pool(name="persist", bufs=1) as persist,
            tc.tile_pool(name="scratch", bufs=3) as scratch,
            tc.tile_pool(name="dscratch", bufs=2) as dscratch,
            tc.tile_pool(name="psum", bufs=5, space="PSUM") as psum,
            tc.tile_pool(name="psumlab", bufs=2, space="PSUM") as psumlab,
            tc.tile_pool(name="psumloss", bufs=1, space="PSUM") as psumloss,
        ):
            xall = persist.tile([P, KC, NB, BLK], BF16, tag="xall")
            labb = persist.tile([P, 640], F32, tag="labb")
            labr = persist.tile([1, 640], BF16, tag="labr")
            augs = persist.tile([2, N], BF16, tag="augs")
            rd = persist.tile([P, 4 * 3], F32, tag="rd")
            coefs = persist.tile([4, 1], F32, tag="coefs")
            ones2 = persist.tile([2, P], BF16, tag="ones2")
            ones1f = persist.tile([P, 1], F32, tag="ones1f")
            acc = persist.tile([P, ACC_W], F32, tag="acc")
            g4 = persist.tile([P, 4], F32, tag="g4")
            c4 = persist.tile([4, 1], F32, tag="c4")
            lossb = persist.tile([1, 1], F32, tag="lossb")

            nc.sync.dma_start(out=rd[:], in_=rowd_d[:])
            nc.sync.dma_start(out=coefs[:], in_=coef_d[:])
            nc.sync.dma_start(out=augs[:], in_=aug_d[:])
            nc.sync.dma_start(out=labr[:], in_=lab_d[:])
            for s in range(NB):
                nc.sync.dma_start(
                    out=xall[:, :, s, :],
                    in_=xtp_d[s].rearrange("(kc p) n -> p kc n", p=P),
                )

            nc.gpsimd.memset(ones2[:], 1.0)
            nc.gpsimd.memset(ones1f[:], 1.0)
            nc.gpsimd.memset(acc[:], 0.0)

            # broadcast the 640-wide label row across partitions via PE
            for lo, w in ((0, 512), (512, 128)):
                pl = psumlab.tile([P, w], F32, tag="pl")
                nc.tensor.matmul(pl[:], ones2[0:1, :], labr[0:1, lo:lo + w],
                                 start=True, stop=True)
                nc.scalar.activation(labb[:, lo:lo + w], pl[:], AF.Copy)

            for ji, (ls, u, clo, wid, diag) in enumerate(JOBS):
                g = 2 * ls + u
                sq_ap = rd[:, 3 * g + 0:3 * g + 1]
                lb_ap = rd[:, 3 * g + 1:3 * g + 2]
                th_ap = rd[:, 3 * g + 2:3 * g + 3]

                t = psum.tile([P, wid], F32, tag="gram")
                ns = wid // BLK
                s0 = clo // BLK
                for kc in range(KC):
                    nc.tensor.matmul(
                        t[:],
                        xall[:, kc, ls, 128 * u:128 * (u + 1)],
                        xall[:, kc, s0:s0 + ns, :],
                        start=(kc == 0), stop=False,
                    )
                nc.tensor.matmul(t[:], ones2[:, :],
                                 augs[:, clo:clo + wid],
                                 start=False, stop=True)

                if not diag:
                    Lt = scratch.tile([P, wid], F32, tag="L")
                    nc.scalar.activation(Lt[:], t[:], AF.Ln,
                                         bias=sq_ap, scale=-2.0,
                                         accum_out=acc[:, COL_L[ji]:
                                                       COL_L[ji] + 1])
                    nc.vector.tensor_reduce(
                        acc[:, COL_T[ji]:COL_T[ji] + 1], t[:],
                        axis=mybir.AxisListType.X, op=ALU.add)
                    if ji in CORNER_JOBS:
                        # same-label corner vs the consecutive block
                        labwin = labb[:, 256:384] if clo == 256 else \
                            labb[:, 512:640]
                        cw = CORNER_W
                        m = dscratch.tile([P, cw], F32, tag="mc")
                        nc.vector.tensor_scalar(m[:], labwin, lb_ap, None,
                                                ALU.is_equal)
                        prod = dscratch.tile([P, 2 * cw], F32, tag="pc")
                        nc.vector.tensor_tensor(prod[:, 0:cw], m[:],
                                                Lt[:, 0:cw], ALU.mult)
                        nc.vector.tensor_tensor(prod[:, cw:2 * cw], m[:],
                                                t[:, 0:cw], ALU.mult)
                        nc.vector.tensor_reduce(
                            acc[:, COL_ML[ji]:COL_ML[ji] + 1],
                            prod[:, 0:cw], axis=mybir.AxisListType.X,
                            op=ALU.add)
                        nc.vector.tensor_reduce(
                            acc[:, COL_MT[ji]:COL_MT[ji] + 1],
                            prod[:, cw:2 * cw], axis=mybir.AxisListType.X,
                            op=ALU.add)
                else:
                    # clamp (protects the exact diagonal), log, strict-upper
                    t2 = dscratch.tile([P, wid], F32, tag="t2")
                    nc.vector.tensor_scalar(t2[:], t[:], th_ap, None, ALU.min)
                    Lt = scratch.tile([P, wid], F32, tag="L")
                    nc.scalar.activation(Lt[:], t2[:], AF.Ln,
                                         bias=sq_ap, scale=-2.0)
                    up = dscratch.tile([P, 2 * wid], F32, tag="up")
                    for src, off in ((Lt, 0), (t2, wid)):
                        nc.gpsimd.affine_select(
                            out=up[:, off:off + wid], in_=src[:],
                            compare_op=ALU.is_gt, fill=0.0,
                            base=-128 * u, channel_multiplier=-1,
                            pattern=[[1, wid]],
                        )
                    nc.vector.tensor_reduce(
                        acc[:, COL_L[ji]:COL_L[ji] + 1], up[:, 0:wid],
                        axis=mybir.AxisListType.X, op=ALU.add)
                    nc.vector.tensor_reduce(
                        acc[:, COL_T[ji]:COL_T[ji] + 1], up[:, wid:2 * wid],
                        axis=mybir.AxisListType.X, op=ALU.add)
                    # same-label correction, strict upper only
                    labwin = labb[:, 256 * ls:256 * ls + wid]
                    m = dscratch.tile([P, wid], F32, tag="md")
                    nc.vector.tensor_scalar(m[:], labwin, lb_ap, None,
                                            ALU.is_equal)
                    mu = dscratch.tile([P, wid], F32, tag="mu")
                    nc.gpsimd.affine_select(
                        out=mu[:], in_=m[:], compare_op=ALU.is_gt, fill=0.0,
                        base=-128 * u, channel_multiplier=-1,
                        pattern=[[1, wid]],
                    )
                    prod = dscratch.tile([P, 2 * wid], F32, tag="pd")
                    nc.vector.tensor_tensor(prod[:, 0:wid], mu[:], Lt[:],
                                            ALU.mult)
                    nc.vector.tensor_tensor(prod[:, wid:2 * wid], mu[:],
                                            t2[:], ALU.mult)
                    nc.vector.tensor_reduce(
                        acc[:, COL_ML[ji]:COL_ML[ji] + 1], prod[:, 0:wid],
                        axis=mybir.AxisListType.X, op=ALU.add)
                    nc.vector.tensor_reduce(
                        acc[:, COL_MT[ji]:COL_MT[ji] + 1],
                        prod[:, wid:2 * wid],
                        axis=mybir.AxisListType.X, op=ALU.add)

            # final: group-reduce acc, weight by coefs via two tiny matmuls
            for k, (lo, hi) in enumerate(GROUPS):
                nc.vector.tensor_reduce(g4[:, k:k + 1], acc[:, lo:hi],
                                        axis=mybir.AxisListType.X, op=ALU.add)
            p4 = psumloss.tile([4, 1], F32, tag="fin")
            nc.tensor.matmul(p4[:], g4[:], ones1f[:], start=True, stop=True)
            nc.scalar.activation(c4[:], p4[:], AF.Copy)
            pls = psumloss.tile([1, 1], F32, tag="fin")
            nc.tensor.matmul(pls[:], c4[:], coefs[:], start=True, stop=True)
            nc.scalar.activation(lossb[:], pls[:], AF.Copy)
            nc.sync.dma_start(out=out_d[:], in_=lossb[:])

    nc.compile()
    _PROG_CACHE["nc"] = nc
    return nc


def _host_prep(outputs, labels):
    """Sort rows by label, build per-core inputs + the host constant."""
    x = np.asarray(outputs, dtype=np.float32)
    lab = np.asarray(labels)
    assert x.shape == (N, D)
    perm = np.argsort(lab, kind="stable")
    xp = x[perm]
    labp = lab[perm].astype(np.float64)

    # label runs (sorted) -> cnt_same(i) = run_end(i) - i - 1
    runs_end = np.empty(N, dtype=np.int64)
    i = 0
    max_run = 0
    while i < N:
        j = i
        while j < N and labp[j] == labp[i]:
            j += 1
        runs_end[i:j] = j
        max_run = max(max_run, j - i)
        i = j
    assert max_run <= CORNER_W, f"label run {max_run} exceeds corner width"
    cnt_same = runs_end - np.arange(N) - 1

    # cnt_main(i) = BLK*outdeg(block) + (BLK-1 - (i % BLK))
    blocks = np.arange(N) // BLK
    outdeg = np.where(blocks % 2 == 0, 8, 7)
    cnt_main = BLK * outdeg + (BLK - 1 - (np.arange(N) % BLK))

    sq = (xp.astype(np.float64) ** 2).sum(axis=1)
    bias_q = LOG_B - (B_C / (2 * C1)) * sq
    bias_w = -LOG_A - LOG_B + ((A_C + B_C) / (2 * C1)) * sq
    host_add = C1 * float((bias_q * cnt_main).sum()
                          + (bias_w * cnt_same).sum())

    xt_bf = np.ascontiguousarray(xp.T).astype(ml_dtypes.bfloat16)   # [D, N]
    neg_half = -0.5 * sq
    hi = neg_half.astype(ml_dtypes.bfloat16)
    lo = (neg_half - hi.astype(np.float64)).astype(ml_dtypes.bfloat16)

    coef = np.asarray(COEFS, dtype=np.float32).reshape(4, 1)

    in_maps = []
    for d in range(NCORES):
        slabs = _core_slabs(d)
        cols = np.concatenate(
            [np.arange(b * BLK, (b + 1) * BLK) for b in slabs])
        xtp = np.ascontiguousarray(
            xt_bf[:, cols].reshape(D, NB, BLK).transpose(1, 0, 2))
        aug = np.stack([hi[cols], lo[cols]])                       # [2, N]
        # label row for slot0(256) | slot1(256) | slot9 first 128
        lcols = np.concatenate([cols[0:512], cols[9 * BLK:9 * BLK + 128]])
        labrow = labp[lcols].astype(ml_dtypes.bfloat16)[None, :]   # [1, 640]

        rowd = np.zeros((P, 4 * 3), dtype=np.float64)
        for g, (slab, u) in enumerate(((0, 0), (0, 1), (1, 0), (1, 1))):
            rows = slabs[slab] * BLK + 128 * u + np.arange(P)
            sqr = sq[rows]
            rowd[:, 3 * g + 0] = sqr
            rowd[:, 3 * g + 1] = labp[rows]
            rowd[:, 3 * g + 2] = (sqr - EPS_D2) / 2.0
        in_maps.append({
            "xtp": xtp,
            "aug": np.ascontiguousarray(aug),
            "lab": np.ascontiguousarray(labrow),
            "rowd": rowd.astype(np.float32),
            "coef": coef,
        })
    return in_maps, host_add


def kernel(**inputs):
    from concourse.bass_utils import run_bass_kernel_spmd
    nc = _build_program()
    in_maps, host_add = _host_prep(inputs["outputs"], inputs["labels"])
    res = run_bass_kernel_spmd(nc, in_maps, core_ids=list(range(NCORES)))
    total = np.float64(host_add)
    for r in res.results:
        total += np.float64(r["out"][0, 0])
    return np.asarray(total, dtype=np.float32)


# revision 8
# speedup vs baseline: 2.7286x; 1.0556x over previous
"""Trainium2 Bass kernel for the MetricLearning pairwise loss.

Reference math:
    d2[i,j] = max(||x_i||^2 + ||x_j||^2 - 2 x_i.x_j, EPS)
    a = d2/(2k)/sigma^2 ; b = d2/(2k)/omega^2 ; c1 = k/2-1
    per_pair = same ? (-c1*log(a) + a/2) : (c1*log(b) - b/2)
    loss = sum_{i<j} per_pair

Per element, with L = log(d2) and t = x_i.x_j - sq_j/2 (so d2 = -2t + sq_i):
    diff_val = c1*L + B*t + c1*bias_q(i),  bias_q = logB - (B/(2c1))*sq_i
    same-diff correction = -2c1*L - (A+B)*t + c1*bias_w(i),
                           bias_w = -logA - logB + ((A+B)/(2c1))*sq_i
    loss = c1*SUM(L) + B*SUM(t)                      [over all pairs]
         - 2c1*SUM_same(L) - (A+B)*SUM_same(t)       [over same-label pairs]
         + c1*(sum_i bias_q(i)*cnt_main(i) + bias_w(i)*cnt_same(i))  [host]

Rows are globally SORTED BY LABEL, so same-label pairs live only within a
block or in the corner between consecutive blocks (label runs < 128 rows).
The main term therefore needs NO label mask at all (ACT accум + one DVE
reduce per tile); the correction runs on 6 small regions per core.

Sharding: 16 row-blocks of 256; the K16 block-pair graph is oriented so
every core owns one even block (8 partners) + one odd block (7 partners)
plus both within-block triangles -> identical SPMD program on all 8 cores,
per-core variation only in input data (slab permutation).
"""

import numpy as np
import ml_dtypes

N = 4096
D = 1024
P = 128
NB = 16          # row blocks
BLK = 256        # rows per block
KC = D // P      # k chunks (8)
NCORES = 8

SIGMA = 0.2
OMEGA = 1.0
K_F = float(N)
C1 = K_F / 2.0 - 1.0                      # 2047
A_C = 1.0 / (2.0 * K_F * SIGMA * SIGMA)   # 1/327.68
B_C = 1.0 / (2.0 * K_F * OMEGA * OMEGA)   # 1/8192
LOG_A = float(np.log(A_C))
LOG_B = float(np.log(B_C))
EPS_D2 = 1e-3   # clamp floor for the (masked-out) diagonal; real d2 >= ~1500

# job := (lhs_slab in {0,1}, unit u in {0,1}, col_lo in slots*BLK, width, diag)
JOBS = []
for _u in (0, 1):
    JOBS.append((0, _u, 0, 256, True))
    JOBS.append((1, _u, 256, 256, True))
for _u in (0, 1):
    for _g in ((256, 512), (768, 512), (1280, 512), (1792, 512)):
        JOBS.append((0, _u, _g[0], _g[1], False))
    for _g in ((2304, 512), (2816, 512), (3328, 512), (3840, 256)):
        JOBS.append((1, _u, _g[0], _g[1], False))
NJOBS = len(JOBS)  # 20

# correction regions: (job_idx, corner?) — diag jobs (0..3) get in-tile
# upper-triangle same-label correction; the two u=1 cross jobs that start
# at slot1 / slot9 get a 128-wide corner correction (consecutive blocks).
DIAG_JOBS = [ji for ji, j in enumerate(JOBS) if j[4]]
CORNER_JOBS = [ji for ji, j in enumerate(JOBS)
               if not j[4] and j[1] == 1 and j[2] in (256, 2304)]
CORNER_W = 128

# acc column map (raw sums; coefficients applied in the final dot)
ACC_W = 64
COL_L = {ji: ji for ji in range(NJOBS)}              # 0..19   coeff c1
COL_T = {ji: 20 + ji for ji in range(NJOBS)}         # 20..39  coeff B
_corr = DIAG_JOBS + CORNER_JOBS
COL_ML = {ji: 40 + k for k, ji in enumerate(_corr)}  # 40..45  coeff -2c1
COL_MT = {ji: 48 + k for k, ji in enumerate(_corr)}  # 48..53  coeff -(A+B)
COEFS = [C1, B_C, -2.0 * C1, -(A_C + B_C)]           # per group of 16 cols
GROUPS = [(0, 20), (20, 40), (40, 46), (48, 54)]

# emission order: DMA-arrival aligned, diag (heavy-epilogue) jobs interleaved
JOB_ORDER = [0, 4, 2, 12, 5, 1, 13, 6, 3, 14, 7, 15, 8, 16, 9, 17, 10, 18,
             11, 19]


def _partners(d):
    """Block orientation: edge {i,j} (i<j) owned by i if i+j odd else j."""
    l0, l1 = 2 * d, 2 * d + 1
    p8 = [j for j in range(l0 + 1, NB) if j % 2 == 1] + \
         [i for i in range(0, l0) if i % 2 == 0]
    p7 = [j for j in range(l1 + 1, NB) if j % 2 == 0] + \
         [i for i in range(0, l1) if i % 2 == 1]
    assert len(p8) == 8 and len(p7) == 7 and l1 in p8
    return l0, l1, p8, p7


def _core_slabs(d):
    """Slot -> block id (16 slots). slot0=own even, slot1=own odd, and
    slot9 (first partner of the odd block) pinned to block 2d+2 when it
    exists so the consecutive-pair corner lands at a fixed slot."""
    l0, l1, p8, p7 = _partners(d)
    rest8 = [p for p in p8 if p != l1]
    nxt = l1 + 1
    if nxt in p7:
        p7 = [nxt] + [p for p in p7 if p != nxt]
    slabs = [l0, l1] + rest8 + list(p7)
    assert len(slabs) == NB and len(set(slabs)) == NB
    return slabs


_PROG_CACHE = {}


def _build_program():
    if "nc" in _PROG_CACHE:
        return _PROG_CACHE["nc"]
    import concourse.bass as bass  # noqa: F401
    import concourse.bacc as bacc
    import concourse.mybir as mybir
    import concourse.tile as tile

    F32 = mybir.dt.float32
    BF16 = mybir.dt.bfloat16
    AF = mybir.ActivationFunctionType
    ALU = mybir.AluOpType

    nc = bacc.Bacc("TRN2", target_bir_lowering=False, debug=False,
                   num_devices=NCORES)
    xtp_d = nc.dram_tensor("xtp", [NB, D, BLK], BF16, kind="ExternalInput").ap()
    aug_d = nc.dram_tensor("aug", [2, N], BF16, kind="ExternalInput").ap()
    lab_d = nc.dram_tensor("lab", [1, 640], BF16, kind="ExternalInput").ap()
    rowd_d = nc.dram_tensor("rowd", [P, 4 * 3], F32, kind="ExternalInput").ap()
    coef_d = nc.dram_tensor("coef", [4, 1], F32, kind="ExternalInput").ap()
    out_d = nc.dram_tensor("out", [1, 1], F32, kind="ExternalOutput").ap()

    with tile.TileContext(nc) as tc:
        with (
            tc.tile_pool(name="persist", bufs=1) as persist,
            tc.tile_pool(name="scratch", bufs=3) as scratch,
            tc.tile_pool(name="dscratch", bufs=2) as dscratch,
            tc.tile_pool(name="psum", bufs=6, space="PSUM") as psum,
            tc.tile_pool(name="psumloss", bufs=1, space="PSUM") as psumloss,
        ):
            xall = persist.tile([P, KC, NB, BLK], BF16, tag="xall")
            labb = persist.tile([P, 640], F32, tag="labb")
            labr = persist.tile([1, 640], BF16, tag="labr")
            augs = persist.tile([2, N], BF16, tag="augs")
            rd = persist.tile([P, 4 * 3], F32, tag="rd")
            coefs = persist.tile([4, 1], F32, tag="coefs")
            ones2 = persist.tile([2, P], BF16, tag="ones2")
            ones1f = persist.tile([P, 1], F32, tag="ones1f")
            acc = persist.tile([P, ACC_W], F32, tag="acc")
            g4 = persist.tile([P, 4], F32, tag="g4")
            c4 = persist.tile([4, 1], F32, tag="c4")
            lossb = persist.tile([1, 1], F32, tag="lossb")

            def load_slab(s):
                nc.sync.dma_start(
                    out=xall[:, :, s, :],
                    in_=xtp_d[s].rearrange("(kc p) n -> p kc n", p=P),
                )
            nc.sync.dma_start(out=labr[:], in_=lab_d[:])
            load_slab(0)
            nc.sync.dma_start(out=augs[:], in_=aug_d[:])
            nc.sync.dma_start(out=rd[:], in_=rowd_d[:])
            load_slab(1)
            for s in range(2, NB):
                load_slab(s)
            nc.sync.dma_start(out=coefs[:], in_=coef_d[:])

            nc.gpsimd.memset(ones2[:], 1.0)
            nc.gpsimd.memset(ones1f[:], 1.0)
            nc.gpsimd.memset(acc[:], 0.0)

            def lab_bcast():
                # broadcast the 640-wide label row across partitions via PE
                for lo, w in ((0, 512), (512, 128)):
                    pl = psum.tile([P, w], F32, tag="gram")
                    nc.tensor.matmul(pl[:], ones2[0:1, :],
                                     labr[0:1, lo:lo + w],
                                     start=True, stop=True)
                    nc.scalar.activation(labb[:, lo:lo + w], pl[:], AF.Copy)

            lab_bcast()
            for oi, ji in enumerate(JOB_ORDER):
                ls, u, clo, wid, diag = JOBS[ji]
                g = 2 * ls + u
                sq_ap = rd[:, 3 * g + 0:3 * g + 1]
                lb_ap = rd[:, 3 * g + 1:3 * g + 2]
                th_ap = rd[:, 3 * g + 2:3 * g + 3]

                t = psum.tile([P, wid], F32, tag="gram")
                ns = wid // BLK
                s0 = clo // BLK
                for kc in range(KC):
                    nc.tensor.matmul(
                        t[:],
                        xall[:, kc, ls, 128 * u:128 * (u + 1)],
                        xall[:, kc, s0:s0 + ns, :],
                        start=(kc == 0), stop=False,
                    )
                nc.tensor.matmul(t[:], ones2[:, :],
                                 augs[:, clo:clo + wid],
                                 start=False, stop=True)

                if not diag:
                    Lt = scratch.tile([P, wid], F32, tag="L")
                    nc.scalar.activation(Lt[:], t[:], AF.Ln,
                                         bias=sq_ap, scale=-2.0,
                                         accum_out=acc[:, COL_L[ji]:
                                                       COL_L[ji] + 1])
                    nc.vector.tensor_reduce(
                        acc[:, COL_T[ji]:COL_T[ji] + 1], t[:],
                        axis=mybir.AxisListType.X, op=ALU.add)
                    if ji in CORNER_JOBS:
                        # same-label corner vs the consecutive block
                        labwin = labb[:, 256:384] if clo == 256 else \
                            labb[:, 512:640]
                        cw = CORNER_W
                        m = dscratch.tile([P, cw], F32, tag="mc")
                        nc.vector.tensor_scalar(m[:], labwin, lb_ap, None,
                                                ALU.is_equal)
                        prod = dscratch.tile([P, 2 * cw], F32, tag="pc")
                        nc.vector.tensor_tensor(prod[:, 0:cw], m[:],
                                                Lt[:, 0:cw], ALU.mult)
                        nc.vector.tensor_tensor(prod[:, cw:2 * cw], m[:],
                                                t[:, 0:cw], ALU.mult)
                        nc.vector.tensor_reduce(
                            acc[:, COL_ML[ji]:COL_ML[ji] + 1],
                            prod[:, 0:cw], axis=mybir.AxisListType.X,
                            op=ALU.add)
                        nc.vector.tensor_reduce(
                            acc[:, COL_MT[ji]:COL_MT[ji] + 1],
                            prod[:, cw:2 * cw], axis=mybir.AxisListType.X,
                            op=ALU.add)
                else:
                    # clamp (protects the exact diagonal), log, strict-upper
                    t2 = dscratch.tile([P, wid], F32, tag="t2")
                    nc.vector.tensor_scalar(t2[:], t[:], th_ap, None, ALU.min)
                    Lt = scratch.tile([P, wid], F32, tag="L")
                    nc.scalar.activation(Lt[:], t2[:], AF.Ln,
                                         bias=sq_ap, scale=-2.0)
                    up = dscratch.tile([P, 2 * wid], F32, tag="up")
                    for src, off in ((Lt, 0), (t2, wid)):
                        nc.gpsimd.affine_select(
                            out=up[:, off:off + wid], in_=src[:],
                            compare_op=ALU.is_gt, fill=0.0,
                            base=-128 * u, channel_multiplier=-1,
                            pattern=[[1, wid]],
                        )
                    nc.vector.tensor_reduce(
                        acc[:, COL_L[ji]:COL_L[ji] + 1], up[:, 0:wid],
                        axis=mybir.AxisListType.X, op=ALU.add)
                    nc.vector.tensor_reduce(
                        acc[:, COL_T[ji]:COL_T[ji] + 1], up[:, wid:2 * wid],
                        axis=mybir.AxisListType.X, op=ALU.add)
                    # same-label correction, strict upper only
                    labwin = labb[:, 256 * ls:256 * ls + wid]
                    m = dscratch.tile([P, wid], F32, tag="md")
                    nc.vector.tensor_scalar(m[:], labwin, lb_ap, None,
                                            ALU.is_equal)
                    mu = dscratch.tile([P, wid], F32, tag="mu")
                    nc.gpsimd.affine_select(
                        out=mu[:], in_=m[:], compare_op=ALU.is_gt, fill=0.0,
                        base=-128 * u, channel_multiplier=-1,
                        pattern=[[1, wid]],
                    )
                    prod = dscratch.tile([P, 2 * wid], F32, tag="pd")
                    nc.vector.tensor_tensor(prod[:, 0:wid], mu[:], Lt[:],
                                            ALU.mult)
                    nc.vector.tensor_tensor(prod[:, wid:2 * wid], mu[:],
                                            t2[:], ALU.mult)
                    nc.vector.tensor_reduce(
                        acc[:, COL_ML[ji]:COL_ML[ji] + 1], prod[:, 0:wid],
                        axis=mybir.AxisListType.X, op=ALU.add)
                    nc.vector.tensor_reduce(
                        acc[:, COL_MT[ji]:COL_MT[ji] + 1],
                        prod[:, wid:2 * wid],
                        axis=mybir.AxisListType.X, op=ALU.add)

            # final: group-reduce acc, weight by coefs via two tiny matmuls
            for k, (lo, hi) in enumerate(GROUPS):
                nc.vector.tensor_reduce(g4[:, k:k + 1], acc[:, lo:hi],
                                        axis=mybir.AxisListType.X, op=ALU.add)
            p4 = psumloss.tile([4, 1], F32, tag="fin")
            nc.tensor.matmul(p4[:], g4[:], ones1f[:], start=True, stop=True)
            nc.scalar.activation(c4[:], p4[:], AF.Copy)
            pls = psumloss.tile([1, 1], F32, tag="fin")
            nc.tensor.matmul(pls[:], c4[:], coefs[:], start=True, stop=True)
            nc.scalar.activation(lossb[:], pls[:], AF.Copy)
            nc.sync.dma_start(out=out_d[:], in_=lossb[:])

    nc.compile()
    _PROG_CACHE["nc"] = nc
    return nc


def _host_prep(outputs, labels):
    """Sort rows by label, build per-core inputs + the host constant."""
    x = np.asarray(outputs, dtype=np.float32)
    lab = np.asarray(labels)
    assert x.shape == (N, D)
    perm = np.argsort(lab, kind="stable")
    xp = x[perm]
    labp = lab[perm].astype(np.float64)

    # label runs (sorted) -> cnt_same(i) = run_end(i) - i - 1
    runs_end = np.empty(N, dtype=np.int64)
    i = 0
    max_run = 0
    while i < N:
        j = i
        while j < N and labp[j] == labp[i]:
            j += 1
        runs_end[i:j] = j
        max_run = max(max_run, j - i)
        i = j
    assert max_run <= CORNER_W, f"label run {max_run} exceeds corner width"
    cnt_same = runs_end - np.arange(N) - 1

    # cnt_main(i) = BLK*outdeg(block) + (BLK-1 - (i % BLK))
    blocks = np.arange(N) // BLK
    outdeg = np.where(blocks % 2 == 0, 8, 7)
    cnt_main = BLK * outdeg + (BLK - 1 - (np.arange(N) % BLK))

    sq = (xp.astype(np.float64) ** 2).sum(axis=1)
    bias_q = LOG_B - (B_C / (2 * C1)) * sq
    bias_w = -LOG_A - LOG_B + ((A_C + B_C) / (2 * C1)) * sq
    host_add = C1 * float((bias_q * cnt_main).sum()
                          + (bias_w * cnt_same).sum())

    xt_bf = np.ascontiguousarray(xp.T).astype(ml_dtypes.bfloat16)   # [D, N]
    neg_half = -0.5 * sq
    hi = neg_half.astype(ml_dtypes.bfloat16)
    lo = (neg_half - hi.astype(np.float64)).astype(ml_dtypes.bfloat16)

    coef = np.asarray(COEFS, dtype=np.float32).reshape(4, 1)

    in_maps = []
    for d in range(NCORES):
        slabs = _core_slabs(d)
        cols = np.concatenate(
            [np.arange(b * BLK, (b + 1) * BLK) for b in slabs])
        xtp = np.ascontiguousarray(
            xt_bf[:, cols].reshape(D, NB, BLK).transpose(1, 0, 2))
        aug = np.stack([hi[cols], lo[cols]])                       # [2, N]
        # label row for slot0(256) | slot1(256) | slot9 first 128
        lcols = np.concatenate([cols[0:512], cols[9 * BLK:9 * BLK + 128]])
        labrow = labp[lcols].astype(ml_dtypes.bfloat16)[None, :]   # [1, 640]

        rowd = np.zeros((P, 4 * 3), dtype=np.float64)
        for g, (slab, u) in enumerate(((0, 0), (0, 1), (1, 0), (1, 1))):
            rows = slabs[slab] * BLK + 128 * u + np.arange(P)
            sqr = sq[rows]
            rowd[:, 3 * g + 0] = sqr
            rowd[:, 3 * g + 1] = labp[rows]
            rowd[:, 3 * g + 2] = (sqr - EPS_D2) / 2.0
        in_maps.append({
            "xtp": xtp,
            "aug": np.ascontiguousarray(aug),
            "lab": np.ascontiguousarray(labrow),
            "rowd": rowd.astype(np.float32),
            "coef": coef,
        })
    return in_maps, host_add


def kernel(**inputs):
    from concourse.bass_utils import run_bass_kernel_spmd
    nc = _build_program()
    in_maps, host_add = _host_prep(inputs["outputs"], inputs["labels"])
    res = run_bass_kernel_spmd(nc, in_maps, core_ids=list(range(NCORES)))
    total = np.float64(host_add)
    for r in res.results:
        total += np.float64(r["out"][0, 0])
    return np.asarray(total, dtype=np.float32)


# revision 9
# speedup vs baseline: 2.7981x; 1.0254x over previous
"""Trainium2 Bass kernel for the MetricLearning pairwise loss.

Reference math:
    d2[i,j] = max(||x_i||^2 + ||x_j||^2 - 2 x_i.x_j, EPS)
    a = d2/(2k)/sigma^2 ; b = d2/(2k)/omega^2 ; c1 = k/2-1
    per_pair = same ? (-c1*log(a) + a/2) : (c1*log(b) - b/2)
    loss = sum_{i<j} per_pair

Per element, with L = log(d2) and t = x_i.x_j - sq_j/2 (so d2 = -2t + sq_i):
    diff_val = c1*L + B*t + c1*bias_q(i),  bias_q = logB - (B/(2c1))*sq_i
    same-diff correction = -2c1*L - (A+B)*t + c1*bias_w(i),
                           bias_w = -logA - logB + ((A+B)/(2c1))*sq_i
    loss = c1*SUM(L) + B*SUM(t)                      [over all pairs]
         - 2c1*SUM_same(L) - (A+B)*SUM_same(t)       [over same-label pairs]
         + c1*(sum_i bias_q(i)*cnt_main(i) + bias_w(i)*cnt_same(i))  [host]

Rows are globally SORTED BY LABEL, so same-label pairs live only within a
block or in the corner between consecutive blocks (label runs < 128 rows).
The main term therefore needs NO label mask at all (ACT accум + one DVE
reduce per tile); the correction runs on 6 small regions per core.

Sharding: 16 row-blocks of 256; the K16 block-pair graph is oriented so
every core owns one even block (8 partners) + one odd block (7 partners)
plus both within-block triangles -> identical SPMD program on all 8 cores,
per-core variation only in input data (slab permutation).
"""

import numpy as np
import ml_dtypes

N = 4096
D = 1024
P = 128
NB = 16          # row blocks
BLK = 256        # rows per block
KC = D // P      # k chunks (8)
NCORES = 8

SIGMA = 0.2
OMEGA = 1.0
K_F = float(N)
C1 = K_F / 2.0 - 1.0                      # 2047
A_C = 1.0 / (2.0 * K_F * SIGMA * SIGMA)   # 1/327.68
B_C = 1.0 / (2.0 * K_F * OMEGA * OMEGA)   # 1/8192
LOG_A = float(np.log(A_C))
LOG_B = float(np.log(B_C))
EPS_D2 = 1e-3   # clamp floor for the (masked-out) diagonal; real d2 >= ~1500

# job := (lhs_slab in {0,1}, unit u in {0,1}, col_lo in slots*BLK, width, diag)
JOBS = []
for _u in (0, 1):
    JOBS.append((0, _u, 0, 256, True))
    JOBS.append((1, _u, 256, 256, True))
for _u in (0, 1):
    for _g in ((256, 512), (768, 512), (1280, 512), (1792, 512)):
        JOBS.append((0, _u, _g[0], _g[1], False))
    for _g in ((2304, 512), (2816, 512), (3328, 512), (3840, 256)):
        JOBS.append((1, _u, _g[0], _g[1], False))
NJOBS = len(JOBS)  # 20

# correction regions: (job_idx, corner?) — diag jobs (0..3) get in-tile
# upper-triangle same-label correction; the two u=1 cross jobs that start
# at slot1 / slot9 get a 128-wide corner correction (consecutive blocks).
DIAG_JOBS = [ji for ji, j in enumerate(JOBS) if j[4]]
CORNER_JOBS = [ji for ji, j in enumerate(JOBS)
               if not j[4] and j[1] == 1 and j[2] in (256, 2304)]
CORNER_W = 128

# acc column map (raw sums; coefficients applied in the final dot)
ACC_W = 64
COL_L = {ji: ji for ji in range(NJOBS)}              # 0..19   coeff c1
COL_T = {ji: 20 + ji for ji in range(NJOBS)}         # 20..39  coeff B
_corr = DIAG_JOBS + CORNER_JOBS
COL_ML = {ji: 40 + k for k, ji in enumerate(_corr)}  # 40..45  coeff -2c1
COL_MT = {ji: 48 + k for k, ji in enumerate(_corr)}  # 48..53  coeff -(A+B)
COEFS = [C1, B_C, -2.0 * C1, -(A_C + B_C)]           # per group of 16 cols
GROUPS = [(0, 20), (20, 40), (40, 46), (48, 54)]

# emission order: DMA-arrival aligned, diag (heavy-epilogue) jobs interleaved
JOB_ORDER = [0, 4, 2, 12, 5, 1, 13, 6, 3, 14, 7, 15, 8, 16, 9, 17, 10, 18,
             11, 19]


def _partners(d):
    """Block orientation: edge {i,j} (i<j) owned by i if i+j odd else j."""
    l0, l1 = 2 * d, 2 * d + 1
    p8 = [j for j in range(l0 + 1, NB) if j % 2 == 1] + \
         [i for i in range(0, l0) if i % 2 == 0]
    p7 = [j for j in range(l1 + 1, NB) if j % 2 == 0] + \
         [i for i in range(0, l1) if i % 2 == 1]
    assert len(p8) == 8 and len(p7) == 7 and l1 in p8
    return l0, l1, p8, p7


def _core_slabs(d):
    """Slot -> block id (16 slots). slot0=own even, slot1=own odd, and
    slot9 (first partner of the odd block) pinned to block 2d+2 when it
    exists so the consecutive-pair corner lands at a fixed slot."""
    l0, l1, p8, p7 = _partners(d)
    rest8 = [p for p in p8 if p != l1]
    nxt = l1 + 1
    if nxt in p7:
        p7 = [nxt] + [p for p in p7 if p != nxt]
    slabs = [l0, l1] + rest8 + list(p7)
    assert len(slabs) == NB and len(set(slabs)) == NB
    return slabs


_PROG_CACHE = {}


def _build_program():
    if "nc" in _PROG_CACHE:
        return _PROG_CACHE["nc"]
    import concourse.bass as bass  # noqa: F401
    import concourse.bacc as bacc
    import concourse.mybir as mybir
    import concourse.tile as tile

    F32 = mybir.dt.float32
    BF16 = mybir.dt.bfloat16
    AF = mybir.ActivationFunctionType
    ALU = mybir.AluOpType

    nc = bacc.Bacc("TRN2", target_bir_lowering=False, debug=False,
                   num_devices=NCORES)
    xtp_d = nc.dram_tensor("xtp", [NB, P, KC, BLK], BF16,
                           kind="ExternalInput").ap()
    aug_d = nc.dram_tensor("aug", [2, N], BF16, kind="ExternalInput").ap()
    lab_d = nc.dram_tensor("lab", [1, 640], BF16, kind="ExternalInput").ap()
    rowd_d = nc.dram_tensor("rowd", [P, 4 * 3], F32, kind="ExternalInput").ap()
    coef_d = nc.dram_tensor("coef", [4, 1], F32, kind="ExternalInput").ap()
    out_d = nc.dram_tensor("out", [1, 1], F32, kind="ExternalOutput").ap()

    with tile.TileContext(nc) as tc:
        with (
            tc.tile_pool(name="persist", bufs=1) as persist,
            tc.tile_pool(name="scratch", bufs=3) as scratch,
            tc.tile_pool(name="dscratch", bufs=2) as dscratch,
            tc.tile_pool(name="psum", bufs=6, space="PSUM") as psum,
            tc.tile_pool(name="psumloss", bufs=1, space="PSUM") as psumloss,
        ):
            xall = persist.tile([P, KC, NB, BLK], BF16, tag="xall")
            labb = persist.tile([P, 640], F32, tag="labb")
            labr = persist.tile([1, 640], BF16, tag="labr")
            augs = persist.tile([2, N], BF16, tag="augs")
            rd = persist.tile([P, 4 * 3], F32, tag="rd")
            coefs = persist.tile([4, 1], F32, tag="coefs")
            ones2 = persist.tile([2, P], BF16, tag="ones2")
            ones1f = persist.tile([P, 1], F32, tag="ones1f")
            acc = persist.tile([P, ACC_W], F32, tag="acc")
            g4 = persist.tile([P, 4], F32, tag="g4")
            c4 = persist.tile([4, 1], F32, tag="c4")
            lossb = persist.tile([1, 1], F32, tag="lossb")

            def load_slab(s):
                nc.sync.dma_start(out=xall[:, :, s, :], in_=xtp_d[s])
            nc.scalar.dma_start(out=labr[:], in_=lab_d[:])
            nc.scalar.dma_start(out=augs[:], in_=aug_d[:])
            nc.scalar.dma_start(out=rd[:], in_=rowd_d[:])
            nc.scalar.dma_start(out=coefs[:], in_=coef_d[:])
            for s in range(NB):
                load_slab(s)

            nc.gpsimd.memset(ones2[:], 1.0)
            nc.gpsimd.memset(ones1f[:], 1.0)
            nc.gpsimd.memset(acc[:], 0.0)

            def lab_bcast():
                # broadcast the 640-wide label row across partitions via PE
                for lo, w in ((0, 512), (512, 128)):
                    pl = psum.tile([P, w], F32, tag="gram")
                    nc.tensor.matmul(pl[:], ones2[0:1, :],
                                     labr[0:1, lo:lo + w],
                                     start=True, stop=True)
                    nc.scalar.activation(labb[:, lo:lo + w], pl[:], AF.Copy)

            lab_bcast()
            for oi, ji in enumerate(JOB_ORDER):
                ls, u, clo, wid, diag = JOBS[ji]
                g = 2 * ls + u
                sq_ap = rd[:, 3 * g + 0:3 * g + 1]
                lb_ap = rd[:, 3 * g + 1:3 * g + 2]
                th_ap = rd[:, 3 * g + 2:3 * g + 3]

                t = psum.tile([P, wid], F32, tag="gram")
                ns = wid // BLK
                s0 = clo // BLK
                for kc in range(KC):
                    nc.tensor.matmul(
                        t[:],
                        xall[:, kc, ls, 128 * u:128 * (u + 1)],
                        xall[:, kc, s0:s0 + ns, :],
                        start=(kc == 0), stop=False,
                    )
                nc.tensor.matmul(t[:], ones2[:, :],
                                 augs[:, clo:clo + wid],
                                 start=False, stop=True)

                if not diag:
                    Lt = scratch.tile([P, wid], F32, tag="L")
                    nc.scalar.activation(Lt[:], t[:], AF.Ln,
                                         bias=sq_ap, scale=-2.0,
                                         accum_out=acc[:, COL_L[ji]:
                                                       COL_L[ji] + 1])
                    nc.vector.tensor_reduce(
                        acc[:, COL_T[ji]:COL_T[ji] + 1], t[:],
                        axis=mybir.AxisListType.X, op=ALU.add)
                    if ji in CORNER_JOBS:
                        # same-label corner vs the consecutive block
                        labwin = labb[:, 256:384] if clo == 256 else \
                            labb[:, 512:640]
                        cw = CORNER_W
                        m = dscratch.tile([P, cw], F32, tag="mc")
                        nc.vector.tensor_scalar(m[:], labwin, lb_ap, None,
                                                ALU.is_equal)
                        prod = dscratch.tile([P, 2 * cw], F32, tag="pc")
                        nc.vector.tensor_tensor(prod[:, 0:cw], m[:],
                                                Lt[:, 0:cw], ALU.mult)
                        nc.vector.tensor_tensor(prod[:, cw:2 * cw], m[:],
                                                t[:, 0:cw], ALU.mult)
                        nc.vector.tensor_reduce(
                            acc[:, COL_ML[ji]:COL_ML[ji] + 1],
                            prod[:, 0:cw], axis=mybir.AxisListType.X,
                            op=ALU.add)
                        nc.vector.tensor_reduce(
                            acc[:, COL_MT[ji]:COL_MT[ji] + 1],
                            prod[:, cw:2 * cw], axis=mybir.AxisListType.X,
                            op=ALU.add)
                else:
                    # clamp (protects the exact diagonal), log, strict-upper
                    t2 = dscratch.tile([P, wid], F32, tag="t2")
                    nc.vector.tensor_scalar(t2[:], t[:], th_ap, None, ALU.min)
                    Lt = scratch.tile([P, wid], F32, tag="L")
                    nc.scalar.activation(Lt[:], t2[:], AF.Ln,
                                         bias=sq_ap, scale=-2.0)
                    up = dscratch.tile([P, 2 * wid], F32, tag="up")
                    for src, off in ((Lt, 0), (t2, wid)):
                        nc.gpsimd.affine_select(
                            out=up[:, off:off + wid], in_=src[:],
                            compare_op=ALU.is_gt, fill=0.0,
                            base=-128 * u, channel_multiplier=-1,
                            pattern=[[1, wid]],
                        )
                    nc.vector.tensor_reduce(
                        acc[:, COL_L[ji]:COL_L[ji] + 1], up[:, 0:wid],
                        axis=mybir.AxisListType.X, op=ALU.add)
                    nc.vector.tensor_reduce(
                        acc[:, COL_T[ji]:COL_T[ji] + 1], up[:, wid:2 * wid],
                        axis=mybir.AxisListType.X, op=ALU.add)
                    # same-label correction, strict upper only
                    labwin = labb[:, 256 * ls:256 * ls + wid]
                    m = dscratch.tile([P, wid], F32, tag="md")
                    nc.vector.tensor_scalar(m[:], labwin, lb_ap, None,
                                            ALU.is_equal)
                    mu = dscratch.tile([P, wid], F32, tag="mu")
                    nc.gpsimd.affine_select(
                        out=mu[:], in_=m[:], compare_op=ALU.is_gt, fill=0.0,
                        base=-128 * u, channel_multiplier=-1,
                        pattern=[[1, wid]],
                    )
                    prod = dscratch.tile([P, 2 * wid], F32, tag="pd")
                    nc.vector.tensor_tensor(prod[:, 0:wid], mu[:], Lt[:],
                                            ALU.mult)
                    nc.vector.tensor_tensor(prod[:, wid:2 * wid], mu[:],
                                            t2[:], ALU.mult)
                    nc.vector.tensor_reduce(
                        acc[:, COL_ML[ji]:COL_ML[ji] + 1], prod[:, 0:wid],
                        axis=mybir.AxisListType.X, op=ALU.add)
                    nc.vector.tensor_reduce(
                        acc[:, COL_MT[ji]:COL_MT[ji] + 1],
                        prod[:, wid:2 * wid],
                        axis=mybir.AxisListType.X, op=ALU.add)

            # final: group-reduce acc, weight by coefs via two tiny matmuls
            for k, (lo, hi) in enumerate(GROUPS):
                nc.vector.tensor_reduce(g4[:, k:k + 1], acc[:, lo:hi],
                                        axis=mybir.AxisListType.X, op=ALU.add)
            p4 = psumloss.tile([4, 1], F32, tag="fin")
            nc.tensor.matmul(p4[:], g4[:], ones1f[:], start=True, stop=True)
            nc.scalar.activation(c4[:], p4[:], AF.Copy)
            pls = psumloss.tile([1, 1], F32, tag="fin")
            nc.tensor.matmul(pls[:], c4[:], coefs[:], start=True, stop=True)
            nc.scalar.activation(lossb[:], pls[:], AF.Copy)
            nc.sync.dma_start(out=out_d[:], in_=lossb[:])

    nc.compile()
    _PROG_CACHE["nc"] = nc
    return nc


def _host_prep(outputs, labels):
    """Sort rows by label, build per-core inputs + the host constant."""
    x = np.asarray(outputs, dtype=np.float32)
    lab = np.asarray(labels)
    assert x.shape == (N, D)
    perm = np.argsort(lab, kind="stable")
    xp = x[perm]
    labp = lab[perm].astype(np.float64)

    # label runs (sorted) -> cnt_same(i) = run_end(i) - i - 1
    runs_end = np.empty(N, dtype=np.int64)
    i = 0
    max_run = 0
    while i < N:
        j = i
        while j < N and labp[j] == labp[i]:
            j += 1
        runs_end[i:j] = j
        max_run = max(max_run, j - i)
        i = j
    assert max_run <= CORNER_W, f"label run {max_run} exceeds corner width"
    cnt_same = runs_end - np.arange(N) - 1

    # cnt_main(i) = BLK*outdeg(block) + (BLK-1 - (i % BLK))
    blocks = np.arange(N) // BLK
    outdeg = np.where(blocks % 2 == 0, 8, 7)
    cnt_main = BLK * outdeg + (BLK - 1 - (np.arange(N) % BLK))

    sq = (xp.astype(np.float64) ** 2).sum(axis=1)
    bias_q = LOG_B - (B_C / (2 * C1)) * sq
    bias_w = -LOG_A - LOG_B + ((A_C + B_C) / (2 * C1)) * sq
    host_add = C1 * float((bias_q * cnt_main).sum()
                          + (bias_w * cnt_same).sum())

    xt_bf = np.ascontiguousarray(xp.T).astype(ml_dtypes.bfloat16)   # [D, N]
    neg_half = -0.5 * sq
    hi = neg_half.astype(ml_dtypes.bfloat16)
    lo = (neg_half - hi.astype(np.float64)).astype(ml_dtypes.bfloat16)

    coef = np.asarray(COEFS, dtype=np.float32).reshape(4, 1)

    in_maps = []
    for d in range(NCORES):
        slabs = _core_slabs(d)
        cols = np.concatenate(
            [np.arange(b * BLK, (b + 1) * BLK) for b in slabs])
        xtp = np.ascontiguousarray(
            xt_bf[:, cols].reshape(KC, P, NB, BLK).transpose(2, 1, 0, 3))
        aug = np.stack([hi[cols], lo[cols]])                       # [2, N]
        # label row for slot0(256) | slot1(256) | slot9 first 128
        lcols = np.concatenate([cols[0:512], cols[9 * BLK:9 * BLK + 128]])
        labrow = labp[lcols].astype(ml_dtypes.bfloat16)[None, :]   # [1, 640]

        rowd = np.zeros((P, 4 * 3), dtype=np.float64)
        for g, (slab, u) in enumerate(((0, 0), (0, 1), (1, 0), (1, 1))):
            rows = slabs[slab] * BLK + 128 * u + np.arange(P)
            sqr = sq[rows]
            rowd[:, 3 * g + 0] = sqr
            rowd[:, 3 * g + 1] = labp[rows]
            rowd[:, 3 * g + 2] = (sqr - EPS_D2) / 2.0
        in_maps.append({
            "xtp": xtp,
            "aug": np.ascontiguousarray(aug),
            "lab": np.ascontiguousarray(labrow),
            "rowd": rowd.astype(np.float32),
            "coef": coef,
        })
    return in_maps, host_add


def kernel(**inputs):
    from concourse.bass_utils import run_bass_kernel_spmd
    nc = _build_program()
    in_maps, host_add = _host_prep(inputs["outputs"], inputs["labels"])
    res = run_bass_kernel_spmd(nc, in_maps, core_ids=list(range(NCORES)))
    total = np.float64(host_add)
    for r in res.results:
        total += np.float64(r["out"][0, 0])
    return np.asarray(total, dtype=np.float32)


# revision 10
# speedup vs baseline: 3.0263x; 1.0816x over previous
"""Trainium2 Bass kernel for the MetricLearning pairwise loss.

Reference math:
    d2[i,j] = max(||x_i||^2 + ||x_j||^2 - 2 x_i.x_j, EPS)
    a = d2/(2k)/sigma^2 ; b = d2/(2k)/omega^2 ; c1 = k/2-1
    per_pair = same ? (-c1*log(a) + a/2) : (c1*log(b) - b/2)
    loss = sum_{i<j} per_pair

Per element, with L = log(d2) and t = x_i.x_j - sq_j/2 (so d2 = -2t + sq_i):
    diff_val = c1*L + B*t + c1*bias_q(i),  bias_q = logB - (B/(2c1))*sq_i
    same-diff correction = -2c1*L - (A+B)*t + c1*bias_w(i),
                           bias_w = -logA - logB + ((A+B)/(2c1))*sq_i
    loss = c1*SUM(L) + B*SUM(t)                      [over all pairs]
         - 2c1*SUM_same(L) - (A+B)*SUM_same(t)       [over same-label pairs]
         + c1*(sum_i bias_q(i)*cnt_main(i) + bias_w(i)*cnt_same(i))  [host]

Rows are globally SORTED BY LABEL, so same-label pairs live only within a
block or in the corner between consecutive blocks (label runs < 128 rows).
The main term therefore needs NO label mask at all (ACT accум + one DVE
reduce per tile); the correction runs on 6 small regions per core.

Sharding: 16 row-blocks of 256; the K16 block-pair graph is oriented so
every core owns one even block (8 partners) + one odd block (7 partners)
plus both within-block triangles -> identical SPMD program on all 8 cores,
per-core variation only in input data (slab permutation).
"""

import numpy as np
import ml_dtypes

N = 4096
D = 1024
P = 128
NB = 16          # row blocks
BLK = 256        # rows per block
KC = D // P      # k chunks (8)
NCORES = 8

SIGMA = 0.2
OMEGA = 1.0
K_F = float(N)
C1 = K_F / 2.0 - 1.0                      # 2047
A_C = 1.0 / (2.0 * K_F * SIGMA * SIGMA)   # 1/327.68
B_C = 1.0 / (2.0 * K_F * OMEGA * OMEGA)   # 1/8192
LOG_A = float(np.log(A_C))
LOG_B = float(np.log(B_C))
EPS_D2 = 1e-3   # clamp floor for the (masked-out) diagonal; real d2 >= ~1500

# job := (lhs_slab in {0,1}, unit u in {0,1}, col_lo in slots*BLK, width, diag)
JOBS = []
for _u in (0, 1):
    JOBS.append((0, _u, 0, 256, True))
    JOBS.append((1, _u, 256, 256, True))
for _u in (0, 1):
    for _g in ((256, 512), (768, 512), (1280, 512), (1792, 512)):
        JOBS.append((0, _u, _g[0], _g[1], False))
    for _g in ((2304, 512), (2816, 512), (3328, 512), (3840, 256)):
        JOBS.append((1, _u, _g[0], _g[1], False))
NJOBS = len(JOBS)  # 20

# correction regions: (job_idx, corner?) — diag jobs (0..3) get in-tile
# upper-triangle same-label correction; the two u=1 cross jobs that start
# at slot1 / slot9 get a 128-wide corner correction (consecutive blocks).
DIAG_JOBS = [ji for ji, j in enumerate(JOBS) if j[4]]
CORNER_JOBS = [ji for ji, j in enumerate(JOBS)
               if not j[4] and j[1] == 1 and j[2] in (256, 2304)]
CORNER_W = 128

# acc column map (raw sums; coefficients applied in the final dot)
ACC_W = 64
COL_L = {ji: ji for ji in range(NJOBS)}              # 0..19   coeff c1
COL_T = {ji: 20 + ji for ji in range(NJOBS)}         # 20..39  coeff B
_corr = DIAG_JOBS + CORNER_JOBS
COL_ML = {ji: 40 + k for k, ji in enumerate(_corr)}  # 40..45  coeff -2c1
COL_MT = {ji: 48 + k for k, ji in enumerate(_corr)}  # 48..53  coeff -(A+B)
COEFS = [C1, B_C, -2.0 * C1, -(A_C + B_C)]           # per group of 16 cols
GROUPS = [(0, 20), (20, 40), (40, 46), (48, 54)]

# emission order: DMA-arrival aligned, diag (heavy-epilogue) jobs interleaved
JOB_ORDER = [0, 2, 1, 3, 4, 12, 5, 13, 6, 14, 7, 15, 8, 16, 9, 17, 10, 18,
             11, 19]


def _partners(d):
    """Block orientation: edge {i,j} (i<j) owned by i if i+j odd else j."""
    l0, l1 = 2 * d, 2 * d + 1
    p8 = [j for j in range(l0 + 1, NB) if j % 2 == 1] + \
         [i for i in range(0, l0) if i % 2 == 0]
    p7 = [j for j in range(l1 + 1, NB) if j % 2 == 0] + \
         [i for i in range(0, l1) if i % 2 == 1]
    assert len(p8) == 8 and len(p7) == 7 and l1 in p8
    return l0, l1, p8, p7


def _core_slabs(d):
    """Slot -> block id (16 slots). slot0=own even, slot1=own odd, and
    slot9 (first partner of the odd block) pinned to block 2d+2 when it
    exists so the consecutive-pair corner lands at a fixed slot."""
    l0, l1, p8, p7 = _partners(d)
    rest8 = [p for p in p8 if p != l1]
    nxt = l1 + 1
    if nxt in p7:
        p7 = [nxt] + [p for p in p7 if p != nxt]
    slabs = [l0, l1] + rest8 + list(p7)
    assert len(slabs) == NB and len(set(slabs)) == NB
    return slabs


_PROG_CACHE = {}


def _build_program():
    if "nc" in _PROG_CACHE:
        return _PROG_CACHE["nc"]
    import concourse.bass as bass  # noqa: F401
    import concourse.bacc as bacc
    import concourse.mybir as mybir
    import concourse.tile as tile

    F32 = mybir.dt.float32
    BF16 = mybir.dt.bfloat16
    AF = mybir.ActivationFunctionType
    ALU = mybir.AluOpType

    nc = bacc.Bacc("TRN2", target_bir_lowering=False, debug=False,
                   num_devices=NCORES)
    xtp_d = nc.dram_tensor("xtp", [NB, P, KC, BLK], BF16,
                           kind="ExternalInput").ap()
    aug_d = nc.dram_tensor("aug", [2, N], BF16, kind="ExternalInput").ap()
    lab_d = nc.dram_tensor("lab", [1, 640], BF16, kind="ExternalInput").ap()
    rowd_d = nc.dram_tensor("rowd", [P, 4 * 3], F32, kind="ExternalInput").ap()
    coef_d = nc.dram_tensor("coef", [4, 1], F32, kind="ExternalInput").ap()
    out_d = nc.dram_tensor("out", [1, 1], F32, kind="ExternalOutput").ap()

    with tile.TileContext(nc) as tc:
        with (
            tc.tile_pool(name="persist", bufs=1) as persist,
            tc.tile_pool(name="scratch", bufs=3) as scratch,
            tc.tile_pool(name="dscratch", bufs=2) as dscratch,
            tc.tile_pool(name="psum", bufs=6, space="PSUM") as psum,
            tc.tile_pool(name="psumloss", bufs=1, space="PSUM") as psumloss,
        ):
            xall = persist.tile([P, KC, NB, BLK], BF16, tag="xall")
            labb = persist.tile([P, 640], F32, tag="labb")
            labr = persist.tile([1, 640], BF16, tag="labr")
            augs = persist.tile([2, N], BF16, tag="augs")
            rd = persist.tile([P, 4 * 3], F32, tag="rd")
            coefs = persist.tile([4, 1], F32, tag="coefs")
            ones2 = persist.tile([2, P], BF16, tag="ones2")
            ones1f = persist.tile([P, 1], F32, tag="ones1f")
            acc = persist.tile([P, ACC_W], F32, tag="acc")
            g4 = persist.tile([P, 4], F32, tag="g4")
            c4 = persist.tile([4, 1], F32, tag="c4")
            lossb = persist.tile([1, 1], F32, tag="lossb")

            def load_slab(s):
                nc.sync.dma_start(out=xall[:, :, s, :], in_=xtp_d[s])
            nc.scalar.dma_start(out=labr[:], in_=lab_d[:])
            nc.scalar.dma_start(out=augs[:], in_=aug_d[:])
            nc.scalar.dma_start(out=rd[:], in_=rowd_d[:])
            nc.scalar.dma_start(out=coefs[:], in_=coef_d[:])
            for s in range(NB):
                load_slab(s)

            nc.gpsimd.memset(ones2[:], 1.0)
            nc.gpsimd.memset(ones1f[:], 1.0)
            nc.gpsimd.memset(acc[:], 0.0)

            def lab_bcast():
                # broadcast the 640-wide label row across partitions via PE
                for lo, w in ((0, 512), (512, 128)):
                    pl = psum.tile([P, w], F32, tag="gram")
                    nc.tensor.matmul(pl[:], ones2[0:1, :],
                                     labr[0:1, lo:lo + w],
                                     start=True, stop=True)
                    nc.scalar.activation(labb[:, lo:lo + w], pl[:], AF.Copy)

            lab_bcast()
            for oi, ji in enumerate(JOB_ORDER):
                ls, u, clo, wid, diag = JOBS[ji]
                g = 2 * ls + u
                sq_ap = rd[:, 3 * g + 0:3 * g + 1]
                lb_ap = rd[:, 3 * g + 1:3 * g + 2]
                th_ap = rd[:, 3 * g + 2:3 * g + 3]

                t = psum.tile([P, wid], F32, tag="gram")
                ns = wid // BLK
                s0 = clo // BLK
                for kc in range(KC):
                    nc.tensor.matmul(
                        t[:],
                        xall[:, kc, ls, 128 * u:128 * (u + 1)],
                        xall[:, kc, s0:s0 + ns, :],
                        start=(kc == 0), stop=False,
                    )
                nc.tensor.matmul(t[:], ones2[:, :],
                                 augs[:, clo:clo + wid],
                                 start=False, stop=True)

                if not diag:
                    Lt = scratch.tile([P, wid], F32, tag="L")
                    nc.scalar.activation(Lt[:], t[:], AF.Ln,
                                         bias=sq_ap, scale=-2.0,
                                         accum_out=acc[:, COL_L[ji]:
                                                       COL_L[ji] + 1])
                    nc.vector.tensor_reduce(
                        acc[:, COL_T[ji]:COL_T[ji] + 1], t[:],
                        axis=mybir.AxisListType.X, op=ALU.add)
                    if ji in CORNER_JOBS:
                        # same-label corner vs the consecutive block
                        labwin = labb[:, 256:384] if clo == 256 else \
                            labb[:, 512:640]
                        cw = CORNER_W
                        m = dscratch.tile([P, cw], F32, tag="mc")
                        nc.vector.tensor_scalar(m[:], labwin, lb_ap, None,
                                                ALU.is_equal)
                        prod = dscratch.tile([P, 2 * cw], F32, tag="pc")
                        nc.vector.tensor_tensor(prod[:, 0:cw], m[:],
                                                Lt[:, 0:cw], ALU.mult)
                        nc.vector.tensor_tensor(prod[:, cw:2 * cw], m[:],
                                                t[:, 0:cw], ALU.mult)
                        nc.vector.tensor_reduce(
                            acc[:, COL_ML[ji]:COL_ML[ji] + 1],
                            prod[:, 0:cw], axis=mybir.AxisListType.X,
                            op=ALU.add)
                        nc.vector.tensor_reduce(
                            acc[:, COL_MT[ji]:COL_MT[ji] + 1],
                            prod[:, cw:2 * cw], axis=mybir.AxisListType.X,
                            op=ALU.add)
                else:
                    # clamp (protects the exact diagonal), log, strict-upper
                    t2 = dscratch.tile([P, wid], F32, tag="t2")
                    nc.vector.tensor_scalar(t2[:], t[:], th_ap, None, ALU.min)
                    Lt = scratch.tile([P, wid], F32, tag="L")
                    nc.scalar.activation(Lt[:], t2[:], AF.Ln,
                                         bias=sq_ap, scale=-2.0)
                    up = dscratch.tile([P, 2 * wid], F32, tag="up")
                    for src, off in ((Lt, 0), (t2, wid)):
                        nc.gpsimd.affine_select(
                            out=up[:, off:off + wid], in_=src[:],
                            compare_op=ALU.is_gt, fill=0.0,
                            base=-128 * u, channel_multiplier=-1,
                            pattern=[[1, wid]],
                        )
                    nc.vector.tensor_reduce(
                        acc[:, COL_L[ji]:COL_L[ji] + 1], up[:, 0:wid],
                        axis=mybir.AxisListType.X, op=ALU.add)
                    nc.vector.tensor_reduce(
                        acc[:, COL_T[ji]:COL_T[ji] + 1], up[:, wid:2 * wid],
                        axis=mybir.AxisListType.X, op=ALU.add)
                    # same-label correction, strict upper only
                    labwin = labb[:, 256 * ls:256 * ls + wid]
                    m = dscratch.tile([P, wid], F32, tag="md")
                    nc.vector.tensor_scalar(m[:], labwin, lb_ap, None,
                                            ALU.is_equal)
                    mu = dscratch.tile([P, wid], F32, tag="mu")
                    nc.gpsimd.affine_select(
                        out=mu[:], in_=m[:], compare_op=ALU.is_gt, fill=0.0,
                        base=-128 * u, channel_multiplier=-1,
                        pattern=[[1, wid]],
                    )
                    prod = dscratch.tile([P, 2 * wid], F32, tag="pd")
                    nc.vector.tensor_tensor(prod[:, 0:wid], mu[:], Lt[:],
                                            ALU.mult)
                    nc.vector.tensor_tensor(prod[:, wid:2 * wid], mu[:],
                                            t2[:], ALU.mult)
                    nc.vector.tensor_reduce(
                        acc[:, COL_ML[ji]:COL_ML[ji] + 1], prod[:, 0:wid],
                        axis=mybir.AxisListType.X, op=ALU.add)
                    nc.vector.tensor_reduce(
                        acc[:, COL_MT[ji]:COL_MT[ji] + 1],
                        prod[:, wid:2 * wid],
                        axis=mybir.AxisListType.X, op=ALU.add)

            # final: group-reduce acc, weight by coefs via two tiny matmuls
            for k, (lo, hi) in enumerate(GROUPS):
                nc.vector.tensor_reduce(g4[:, k:k + 1], acc[:, lo:hi],
                                        axis=mybir.AxisListType.X, op=ALU.add)
            p4 = psumloss.tile([4, 1], F32, tag="fin")
            nc.tensor.matmul(p4[:], g4[:], ones1f[:], start=True, stop=True)
            nc.scalar.activation(c4[:], p4[:], AF.Copy)
            pls = psumloss.tile([1, 1], F32, tag="fin")
            nc.tensor.matmul(pls[:], c4[:], coefs[:], start=True, stop=True)
            nc.scalar.activation(lossb[:], pls[:], AF.Copy)
            nc.sync.dma_start(out=out_d[:], in_=lossb[:])

    nc.compile()
    _PROG_CACHE["nc"] = nc
    return nc


def _host_prep(outputs, labels):
    """Sort rows by label, build per-core inputs + the host constant."""
    x = np.asarray(outputs, dtype=np.float32)
    lab = np.asarray(labels)
    assert x.shape == (N, D)
    perm = np.argsort(lab, kind="stable")
    xp = x[perm]
    labp = lab[perm].astype(np.float64)

    # label runs (sorted) -> cnt_same(i) = run_end(i) - i - 1
    runs_end = np.empty(N, dtype=np.int64)
    i = 0
    max_run = 0
    while i < N:
        j = i
        while j < N and labp[j] == labp[i]:
            j += 1
        runs_end[i:j] = j
        max_run = max(max_run, j - i)
        i = j
    assert max_run <= CORNER_W, f"label run {max_run} exceeds corner width"
    cnt_same = runs_end - np.arange(N) - 1

    # cnt_main(i) = BLK*outdeg(block) + (BLK-1 - (i % BLK))
    blocks = np.arange(N) // BLK
    outdeg = np.where(blocks % 2 == 0, 8, 7)
    cnt_main = BLK * outdeg + (BLK - 1 - (np.arange(N) % BLK))

    sq = (xp.astype(np.float64) ** 2).sum(axis=1)
    bias_q = LOG_B - (B_C / (2 * C1)) * sq
    bias_w = -LOG_A - LOG_B + ((A_C + B_C) / (2 * C1)) * sq
    host_add = C1 * float((bias_q * cnt_main).sum()
                          + (bias_w * cnt_same).sum())

    xt_bf = np.ascontiguousarray(xp.T).astype(ml_dtypes.bfloat16)   # [D, N]
    neg_half = -0.5 * sq
    hi = neg_half.astype(ml_dtypes.bfloat16)
    lo = (neg_half - hi.astype(np.float64)).astype(ml_dtypes.bfloat16)

    coef = np.asarray(COEFS, dtype=np.float32).reshape(4, 1)

    in_maps = []
    for d in range(NCORES):
        slabs = _core_slabs(d)
        cols = np.concatenate(
            [np.arange(b * BLK, (b + 1) * BLK) for b in slabs])
        xtp = np.ascontiguousarray(
            xt_bf[:, cols].reshape(KC, P, NB, BLK).transpose(2, 1, 0, 3))
        aug = np.stack([hi[cols], lo[cols]])                       # [2, N]
        # label row for slot0(256) | slot1(256) | slot9 first 128
        lcols = np.concatenate([cols[0:512], cols[9 * BLK:9 * BLK + 128]])
        labrow = labp[lcols].astype(ml_dtypes.bfloat16)[None, :]   # [1, 640]

        rowd = np.zeros((P, 4 * 3), dtype=np.float64)
        for g, (slab, u) in enumerate(((0, 0), (0, 1), (1, 0), (1, 1))):
            rows = slabs[slab] * BLK + 128 * u + np.arange(P)
            sqr = sq[rows]
            rowd[:, 3 * g + 0] = sqr
            rowd[:, 3 * g + 1] = labp[rows]
            rowd[:, 3 * g + 2] = (sqr - EPS_D2) / 2.0
        in_maps.append({
            "xtp": xtp,
            "aug": np.ascontiguousarray(aug),
            "lab": np.ascontiguousarray(labrow),
            "rowd": rowd.astype(np.float32),
            "coef": coef,
        })
    return in_maps, host_add


def kernel(**inputs):
    from concourse.bass_utils import run_bass_kernel_spmd
    nc = _build_program()
    in_maps, host_add = _host_prep(inputs["outputs"], inputs["labels"])
    res = run_bass_kernel_spmd(nc, in_maps, core_ids=list(range(NCORES)))
    total = np.float64(host_add)
    for r in res.results:
        total += np.float64(r["out"][0, 0])
    return np.asarray(total, dtype=np.float32)


# revision 14
# speedup vs baseline: 3.7812x; 1.2494x over previous
"""Trainium2 Bass kernel for the MetricLearning pairwise loss.

Reference math:
    d2[i,j] = max(||x_i||^2 + ||x_j||^2 - 2 x_i.x_j, EPS)
    a = d2/(2k)/sigma^2 ; b = d2/(2k)/omega^2 ; c1 = k/2-1
    per_pair = same ? (-c1*log(a) + a/2) : (c1*log(b) - b/2)
    loss = sum_{i<j} per_pair

Per element, with L = log(d2) and t = x_i.x_j - sq_j/2 (so d2 = -2t + sq_i):
    diff_val = c1*L + B*t + c1*bias_q(i),  bias_q = logB - (B/(2c1))*sq_i
    same-diff correction = -2c1*L - (A+B)*t + c1*bias_w(i),
                           bias_w = -logA - logB + ((A+B)/(2c1))*sq_i
    loss = c1*SUM(L) + B*SUM(t)                      [over all pairs]
         - 2c1*SUM_same(L) - (A+B)*SUM_same(t)       [over same-label pairs]
         + c1*(sum_i bias_q(i)*cnt_main(i) + bias_w(i)*cnt_same(i))  [host]

Rows are globally SORTED BY LABEL, so same-label pairs live only within a
block or in the corner between consecutive blocks (label runs < 128 rows).
The main term therefore needs NO label mask at all (ACT accум + one DVE
reduce per tile); the correction runs on 6 small regions per core.

Sharding: 16 row-blocks of 256; the K16 block-pair graph is oriented so
every core owns one even block (8 partners) + one odd block (7 partners)
plus both within-block triangles -> identical SPMD program on all 8 cores,
per-core variation only in input data (slab permutation).
"""

import numpy as np
import ml_dtypes

N = 4096
D = 1024
P = 128
NB = 16          # row blocks
BLK = 256        # rows per block
KC = D // P      # k chunks (8)
NCORES = 8

SIGMA = 0.2
OMEGA = 1.0
K_F = float(N)
C1 = K_F / 2.0 - 1.0                      # 2047
A_C = 1.0 / (2.0 * K_F * SIGMA * SIGMA)   # 1/327.68
B_C = 1.0 / (2.0 * K_F * OMEGA * OMEGA)   # 1/8192
LOG_A = float(np.log(A_C))
LOG_B = float(np.log(B_C))
EPS_D2 = 1e-3   # clamp floor for the (masked-out) diagonal; real d2 >= ~1500

# job := (lhs_slab in {0,1}, unit u in {0,1}, col_lo in slots*BLK, width, diag)
JOBS = []
for _u in (0, 1):
    JOBS.append((0, _u, 0, 256, True))
    JOBS.append((1, _u, 256, 256, True))
for _u in (0, 1):
    for _g in ((256, 512), (768, 512), (1280, 512), (1792, 512)):
        JOBS.append((0, _u, _g[0], _g[1], False))
    for _g in ((2304, 512), (2816, 512), (3328, 512), (3840, 256)):
        JOBS.append((1, _u, _g[0], _g[1], False))
NJOBS = len(JOBS)  # 20

# correction regions: (job_idx, corner?) — diag jobs (0..3) get in-tile
# upper-triangle same-label correction; the two u=1 cross jobs that start
# at slot1 / slot9 get a 128-wide corner correction (consecutive blocks).
DIAG_JOBS = [ji for ji, j in enumerate(JOBS) if j[4]]
CORNER_JOBS = [ji for ji, j in enumerate(JOBS)
               if not j[4] and j[1] == 1 and j[2] in (256, 2304)]
CORNER_W = 128

# acc column map (raw sums; coefficients applied in the final dot).
# (L,T) sums sit in adjacent even/odd columns so a single [P,2,w] reduce can
# write both; same for (ML,MT).
ACC_W = 64
COL_L = {ji: 2 * ji for ji in range(NJOBS)}          # even 0..38   coeff c1
COL_T = {ji: 2 * ji + 1 for ji in range(NJOBS)}      # odd  1..39   coeff B
_corr = DIAG_JOBS + CORNER_JOBS
COL_ML = {ji: 40 + 2 * k for k, ji in enumerate(_corr)}      # coeff -2c1
COL_MT = {ji: 41 + 2 * k for k, ji in enumerate(_corr)}      # coeff -(A+B)
COEFS = [C1, B_C, -2.0 * C1, -(A_C + B_C)]
# groups as (start, step, count) over acc columns
GROUPS = [(0, 2, 20), (1, 2, 20), (40, 2, 6), (41, 2, 6)]

# emission order: DMA-arrival aligned, diag (heavy-epilogue) jobs interleaved
JOB_ORDER = [0, 2, 1, 3, 4, 12, 5, 13, 6, 14, 7, 15, 8, 16, 9, 17, 10, 18,
             11, 19]


def _partners(d):
    """Block orientation: edge {i,j} (i<j) owned by i if i+j odd else j."""
    l0, l1 = 2 * d, 2 * d + 1
    p8 = [j for j in range(l0 + 1, NB) if j % 2 == 1] + \
         [i for i in range(0, l0) if i % 2 == 0]
    p7 = [j for j in range(l1 + 1, NB) if j % 2 == 0] + \
         [i for i in range(0, l1) if i % 2 == 1]
    assert len(p8) == 8 and len(p7) == 7 and l1 in p8
    return l0, l1, p8, p7


def _core_slabs(d):
    """Slot -> block id (16 slots). slot0=own even, slot1=own odd, and
    slot9 (first partner of the odd block) pinned to block 2d+2 when it
    exists so the consecutive-pair corner lands at a fixed slot."""
    l0, l1, p8, p7 = _partners(d)
    rest8 = [p for p in p8 if p != l1]
    nxt = l1 + 1
    if nxt in p7:
        p7 = [nxt] + [p for p in p7 if p != nxt]
    slabs = [l0, l1] + rest8 + list(p7)
    assert len(slabs) == NB and len(set(slabs)) == NB
    return slabs


_PROG_CACHE = {}


def _build_program():
    if "nc" in _PROG_CACHE:
        return _PROG_CACHE["nc"]
    import concourse.bass as bass  # noqa: F401
    import concourse.bacc as bacc
    import concourse.mybir as mybir
    import concourse.tile as tile

    F32 = mybir.dt.float32
    BF16 = mybir.dt.bfloat16
    FP8 = mybir.dt.float8e4
    AF = mybir.ActivationFunctionType
    ALU = mybir.AluOpType

    nc = bacc.Bacc("TRN2", target_bir_lowering=False, debug=False,
                   num_devices=NCORES)
    xtp_d = nc.dram_tensor("xtp", [NB, P, KC, BLK], FP8,
                           kind="ExternalInput").ap()
    aug_d = nc.dram_tensor("aug", [2, N], BF16, kind="ExternalInput").ap()
    lab_d = nc.dram_tensor("lab", [1, 640], BF16, kind="ExternalInput").ap()
    rowd_d = nc.dram_tensor("rowd", [P, 4 * 3], F32, kind="ExternalInput").ap()
    coef_d = nc.dram_tensor("coef", [4, 1], F32, kind="ExternalInput").ap()
    out_d = nc.dram_tensor("out", [1, 1], F32, kind="ExternalOutput").ap()

    with tile.TileContext(nc) as tc:
        with (
            tc.tile_pool(name="persist", bufs=1) as persist,
            tc.tile_pool(name="scratch", bufs=3) as scratch,
            tc.tile_pool(name="dscratch", bufs=2) as dscratch,
            tc.tile_pool(name="psum", bufs=6, space="PSUM") as psum,
            tc.tile_pool(name="psumloss", bufs=1, space="PSUM") as psumloss,
        ):
            xall = persist.tile([P, KC, NB, BLK], FP8, tag="xall")
            labb = persist.tile([P, 640], F32, tag="labb")
            labr = persist.tile([1, 640], BF16, tag="labr")
            augs = persist.tile([2, N], BF16, tag="augs")
            rd = persist.tile([P, 4 * 3], F32, tag="rd")
            coefs = persist.tile([4, 1], F32, tag="coefs")
            ones2 = persist.tile([2, P], BF16, tag="ones2")
            ones1f = persist.tile([P, 1], F32, tag="ones1f")
            acc = persist.tile([P, ACC_W], F32, tag="acc")
            g4 = persist.tile([P, 4], F32, tag="g4")
            c4 = persist.tile([4, 1], F32, tag="c4")
            lossb = persist.tile([1, 1], F32, tag="lossb")

            def load_slab(s):
                nc.sync.dma_start(out=xall[:, :, s, :], in_=xtp_d[s])
            nc.scalar.dma_start(out=labr[:], in_=lab_d[:])
            nc.scalar.dma_start(out=augs[:], in_=aug_d[:])
            nc.scalar.dma_start(out=rd[:], in_=rowd_d[:])
            nc.scalar.dma_start(out=coefs[:], in_=coef_d[:])
            for s in range(NB):
                load_slab(s)

            nc.gpsimd.memset(ones2[:], 1.0)
            nc.gpsimd.memset(ones1f[:], 1.0)
            nc.gpsimd.memset(acc[:], 0.0)

            def lab_bcast():
                # broadcast the 640-wide label row across partitions via PE
                for lo, w in ((0, 512), (512, 128)):
                    pl = psum.tile([P, w], F32, tag="gram")
                    nc.tensor.matmul(pl[:], ones2[0:1, :],
                                     labr[0:1, lo:lo + w],
                                     start=True, stop=True)
                    nc.scalar.activation(labb[:, lo:lo + w], pl[:], AF.Copy)

            lab_bcast()
            for oi, ji in enumerate(JOB_ORDER):
                ls, u, clo, wid, diag = JOBS[ji]
                g = 2 * ls + u
                sq_ap = rd[:, 3 * g + 0:3 * g + 1]
                lb_ap = rd[:, 3 * g + 1:3 * g + 2]
                th_ap = rd[:, 3 * g + 2:3 * g + 3]

                t = psum.tile([P, wid], F32, tag="gram")
                ns = wid // BLK
                s0 = clo // BLK
                for kc2 in range(KC // 2):
                    nc.tensor.matmul(
                        t[:],
                        xall[:, 2 * kc2:2 * kc2 + 2, ls,
                             128 * u:128 * (u + 1)],
                        xall[:, 2 * kc2:2 * kc2 + 2, s0:s0 + ns, :],
                        start=(kc2 == 0), stop=False,
                        perf_mode=mybir.MatmulPerfMode.DoubleRow,
                    )
                nc.tensor.matmul(t[:], ones2[:, :],
                                 augs[:, clo:clo + wid],
                                 start=False, stop=True)

                if not diag:
                    Lt = scratch.tile([P, wid], F32, tag="L")
                    nc.scalar.activation(Lt[:], t[:], AF.Ln,
                                         bias=sq_ap, scale=-2.0,
                                         accum_out=acc[:, COL_L[ji]:
                                                       COL_L[ji] + 1])
                    nc.vector.tensor_reduce(
                        acc[:, COL_T[ji]:COL_T[ji] + 1], t[:],
                        axis=mybir.AxisListType.X, op=ALU.add)
                    if ji in CORNER_JOBS:
                        # same-label corner vs the consecutive block
                        labwin = labb[:, 256:384] if clo == 256 else \
                            labb[:, 512:640]
                        cw = CORNER_W
                        m = dscratch.tile([P, cw], F32, tag="mc")
                        nc.vector.tensor_scalar(m[:], labwin, lb_ap, None,
                                                ALU.is_equal)
                        prod = dscratch.tile([P, 2 * cw], F32, tag="pc")
                        nc.vector.tensor_tensor(prod[:, 0:cw], m[:],
                                                Lt[:, 0:cw], ALU.mult)
                        nc.vector.tensor_tensor(prod[:, cw:2 * cw], m[:],
                                                t[:, 0:cw], ALU.mult)
                        nc.vector.tensor_reduce(
                            acc[:, COL_ML[ji]:COL_ML[ji] + 2],
                            prod[:].rearrange("p (two w) -> p two w", two=2),
                            axis=mybir.AxisListType.X, op=ALU.add)
                else:
                    # clamp (protects the exact diagonal), log, strict-upper
                    t2 = dscratch.tile([P, wid], F32, tag="t2")
                    nc.vector.tensor_scalar(t2[:], t[:], th_ap, None, ALU.min)
                    Lt = scratch.tile([P, wid], F32, tag="L")
                    nc.scalar.activation(Lt[:], t2[:], AF.Ln,
                                         bias=sq_ap, scale=-2.0)
                    up = dscratch.tile([P, 2 * wid], F32, tag="up")
                    for src, off in ((Lt, 0), (t2, wid)):
                        nc.gpsimd.affine_select(
                            out=up[:, off:off + wid], in_=src[:],
                            compare_op=ALU.is_gt, fill=0.0,
                            base=-128 * u, channel_multiplier=-1,
                            pattern=[[1, wid]],
                        )
                    nc.vector.tensor_reduce(
                        acc[:, COL_L[ji]:COL_L[ji] + 2],
                        up[:].rearrange("p (two w) -> p two w", two=2),
                        axis=mybir.AxisListType.X, op=ALU.add)
                    # same-label correction, strict upper only
                    labwin = labb[:, 256 * ls:256 * ls + wid]
                    m = dscratch.tile([P, wid], F32, tag="md")
                    nc.vector.tensor_scalar(m[:], labwin, lb_ap, None,
                                            ALU.is_equal)
                    mu = dscratch.tile([P, wid], F32, tag="mu")
                    nc.gpsimd.affine_select(
                        out=mu[:], in_=m[:], compare_op=ALU.is_gt, fill=0.0,
                        base=-128 * u, channel_multiplier=-1,
                        pattern=[[1, wid]],
                    )
                    # mu broadcast over the [L' | t2'] concat: one product
                    prod = dscratch.tile([P, 2 * wid], F32, tag="pd")
                    nc.vector.tensor_tensor(
                        prod[:].rearrange("p (two w) -> p two w", two=2),
                        mu[:].rearrange("p (one w) -> p one w", one=1)
                             .broadcast_to([P, 2, wid]),
                        up[:].rearrange("p (two w) -> p two w", two=2),
                        ALU.mult)
                    nc.vector.tensor_reduce(
                        acc[:, COL_ML[ji]:COL_ML[ji] + 2],
                        prod[:].rearrange("p (two w) -> p two w", two=2),
                        axis=mybir.AxisListType.X, op=ALU.add)

            # final: group-reduce acc, weight by coefs via two tiny matmuls
            for k, (lo, step, cnt) in enumerate(GROUPS):
                nc.vector.tensor_reduce(g4[:, k:k + 1],
                                        acc[:, lo:lo + step * cnt:step],
                                        axis=mybir.AxisListType.X, op=ALU.add)
            p4 = psumloss.tile([4, 1], F32, tag="fin")
            nc.tensor.matmul(p4[:], g4[:], ones1f[:], start=True, stop=True)
            nc.scalar.activation(c4[:], p4[:], AF.Copy)
            pls = psumloss.tile([1, 1], F32, tag="fin")
            nc.tensor.matmul(pls[:], c4[:], coefs[:], start=True, stop=True)
            nc.scalar.activation(lossb[:], pls[:], AF.Copy)
            nc.sync.dma_start(out=out_d[:], in_=lossb[:])

    nc.compile()
    _PROG_CACHE["nc"] = nc
    return nc


def _host_prep(outputs, labels):
    """Sort rows by label, build per-core inputs + the host constant."""
    x = np.asarray(outputs, dtype=np.float32)
    lab = np.asarray(labels)
    assert x.shape == (N, D)
    perm = np.argsort(lab, kind="stable")
    xp = x[perm]
    labp = lab[perm].astype(np.float64)

    # label runs (sorted) -> cnt_same(i) = run_end(i) - i - 1
    runs_end = np.empty(N, dtype=np.int64)
    i = 0
    max_run = 0
    while i < N:
        j = i
        while j < N and labp[j] == labp[i]:
            j += 1
        runs_end[i:j] = j
        max_run = max(max_run, j - i)
        i = j
    assert max_run <= CORNER_W, f"label run {max_run} exceeds corner width"
    cnt_same = runs_end - np.arange(N) - 1

    # cnt_main(i) = BLK*outdeg(block) + (BLK-1 - (i % BLK))
    blocks = np.arange(N) // BLK
    outdeg = np.where(blocks % 2 == 0, 8, 7)
    cnt_main = BLK * outdeg + (BLK - 1 - (np.arange(N) % BLK))

    xq = xp.astype(ml_dtypes.float8_e4m3)
    # True (unquantized) norms make d2 = sq_i + sq_j - 2*xq_i.xq_j unbiased:
    # the value-error correlation in ||xq||^2 cancels the ||e||^2 term.
    sq = (xp.astype(np.float64) ** 2).sum(axis=1)
    bias_q = LOG_B - (B_C / (2 * C1)) * sq
    bias_w = -LOG_A - LOG_B + ((A_C + B_C) / (2 * C1)) * sq
    host_add = C1 * float((bias_q * cnt_main).sum()
                          + (bias_w * cnt_same).sum())

    xt_q = np.ascontiguousarray(xq.T)                               # [D, N]
    neg_half = -0.5 * sq
    hi = neg_half.astype(ml_dtypes.bfloat16)
    lo = (neg_half - hi.astype(np.float64)).astype(ml_dtypes.bfloat16)

    coef = np.asarray(COEFS, dtype=np.float32).reshape(4, 1)

    in_maps = []
    for d in range(NCORES):
        slabs = _core_slabs(d)
        cols = np.concatenate(
            [np.arange(b * BLK, (b + 1) * BLK) for b in slabs])
        xtp = np.ascontiguousarray(
            xt_q[:, cols].reshape(KC, P, NB, BLK).transpose(2, 1, 0, 3))
        aug = np.stack([hi[cols], lo[cols]])                       # [2, N]
        # label row for slot0(256) | slot1(256) | slot9 first 128
        lcols = np.concatenate([cols[0:512], cols[9 * BLK:9 * BLK + 128]])
        labrow = labp[lcols].astype(ml_dtypes.bfloat16)[None, :]   # [1, 640]

        rowd = np.zeros((P, 4 * 3), dtype=np.float64)
        for g, (slab, u) in enumerate(((0, 0), (0, 1), (1, 0), (1, 1))):
            rows = slabs[slab] * BLK + 128 * u + np.arange(P)
            sqr = sq[rows]
            rowd[:, 3 * g + 0] = sqr
            rowd[:, 3 * g + 1] = labp[rows]
            rowd[:, 3 * g + 2] = (sqr - EPS_D2) / 2.0
        in_maps.append({
            "xtp": xtp,
            "aug": np.ascontiguousarray(aug),
            "lab": np.ascontiguousarray(labrow),
            "rowd": rowd.astype(np.float32),
            "coef": coef,
        })
    return in_maps, host_add


def kernel(**inputs):
    from concourse.bass_utils import run_bass_kernel_spmd
    nc = _build_program()
    in_maps, host_add = _host_prep(inputs["outputs"], inputs["labels"])
    res = run_bass_kernel_spmd(nc, in_maps, core_ids=list(range(NCORES)))
    total = np.float64(host_add)
    for r in res.results:
        total += np.float64(r["out"][0, 0])
    return np.asarray(total, dtype=np.float32)
